# revision 21
# baseline (speedup 1.0000x reference)
"""Self-contained Trainium2 kernel for the SSD-scan actor network.

Data-parallel over batch B=8 across 8 NeuronCores (one sample per core, no
collectives). Per core:
  x  = relu(obs @ W_in + b_in)                  (T=512, D=2048)
  dt = softplus(x @ W_dt + dt_bias)             (T, H=16)
  Bm = x @ W_B, Cm = x @ W_C                    (T, H, N=64)
  y  = selective scan over T (Mamba2 SSD)       (T, D)
  z  = relu(y @ W_yo + b_yo)                    (T, U=256)
  out = z @ W_head + b_head                     (T, A=64)

The scan uses the chunked (segsum) SSD formulation: chunk length L=128,
4 chunks, 16 independent heads. Per head/chunk:
  E[j,i]  = exp(Pcum_i - Pcum_j + log dt_j), causally masked to j<=i
  Y^T     = x_chunk^T Gw + S_prev^T (C*u),  Gw = (B C^T)^T . E, u_i=exp(Pcum_i)
  S_new   = exp(Pcum_L-1) S_prev + sum_j exp(Pcum_L-1 - Pcum_j) dt_j B_j x_j^T
Big matmuls run in bf16 with fp32 PSUM accumulation; the Diff matrix
(Pcum_i - Pcum_j + logdt_j) is built exactly with K=4 bf16 (hi/lo) matmuls.

v8 schedule: the PE instruction stream is kept dense end-to-end so the HAM
activity monitor never demotes the clock to 1.2 GHz:
  - weights are pre-transposed on the host so every SBUF load is one
    contiguous descriptor per partition (the old strided loads saturated
    the DMA/sync engines);
  - the dt chain is batched across all 4 chunks (cumsum via the DVE
    tensor_tensor_scan op) so its PE footprint is ~10 small ops that are
    slotted between the B/C projection blocks instead of serializing them;
  - scan groups start as soon as their B/C block lands; the z projection
    and the last head-group's chunk chain share the tail.

Hardware notes (all discovered the hard way on this container's stack):
  - walrus here allows only ONE sync wait per instruction -> _split_multi_waits
  - matmul operands whose APs start at partition 64 crash the exec unit
    (NRT_EXEC_UNIT_UNRECOVERABLE), so every matmul operand is kept at base
    partition 0: B/C are repacked to 64-partition tensors via bf16
    staging + SBUF-to-SBUF DMA (DMA moves across partitions; DVE cannot).
  - Softplus shares no ACT function table with Exp/Ln -> ln(1+exp(x)).
"""

import sys
import types

import numpy as np
import ml_dtypes

T, BSZ, OBSD = 512, 8, 256
D, H, N, P = 2048, 16, 64, 128
U, A = 256, 64
L, NCH, KT = 128, 4, 16  # chunk length, #chunks, #d-tiles (D/128)
MT = 8  # B/C column blocks (HN/128)
BF16 = ml_dtypes.bfloat16

_CACHE = {}
_EXECUTED = {}


def _patch_tile():
    """Split the TileContext final drain's waits across single-wait nops."""
    from concourse import tile, mybir
    from concourse.vector_clock import ScopedClock

    if getattr(tile.TileContext, "_drain_patched", False):
        return

    def _patched(self, tick_clock, wait_clock):
        nc = self.nc
        probe = nc.sync.nop()
        wait_clock.add_sem_waits(
            probe.ins, ScopedClock({None: tick_clock.global_clock})
        )
        si = probe.ins.sync_info
        if si is not None and len(si.on_wait) > 1:
            waits = list(si.on_wait)
            probe.ins.sync_info = mybir.SyncInfo(
                on_wait=[waits[0]], on_update=list(si.on_update)
            )
            for w in waits[1:]:
                nop = nc.sync.nop()
                nop.ins.sync_info = mybir.SyncInfo(on_wait=[w], on_update=[])
        nc.sync.drain()
        nc.all_engine_barrier(sem_only=True)
        assert self.sems is not None
        popped = nc._tile_sem_poison_stack.pop()
        assert popped is self._sem_poison
        nc.clear_and_free_semaphores(list(self.sems.allocated().values()))
        # NOTE: the stock drain runs a second all_engine_barrier here (~5us
        # of ring latency); nothing uses the cleared semaphores afterwards —
        # the NEFF ends — so it is skipped.

    tile.TileContext._drain_and_barrier = _patched
    tile.TileContext._drain_patched = True


def _split_multi_waits(nc):
    """This walrus build accepts at most one sync wait per instruction.
    Hoist extra waits onto single-wait NoOps inserted just before, on the
    same engine (the sequencer stalls there first — strictly conservative)."""
    from concourse import mybir

    n = 0
    for f in nc.m.functions:
        for bb in f.blocks:
            insts = list(bb.instructions)
            changed = False
            new = []
            for inst in insts:
                try:
                    si = inst.sync_info
                except Exception:
                    si = None
                if si is not None and len(si.on_wait) > 1:
                    waits = list(si.on_wait)
                    for w in waits[:-1]:
                        nop = mybir.InstNoOp(
                            name=f"wsplit-{n}", ins=[], outs=[], engine=inst.engine
                        )
                        n += 1
                        nop.sync_info = mybir.SyncInfo(on_wait=[w], on_update=[])
                        nc.register_instruction(nop, overwrite=True)
                        new.append(nop)
                    inst.sync_info = mybir.SyncInfo(
                        on_wait=[waits[-1]], on_update=list(si.on_update)
                    )
                    changed = True
                new.append(inst)
            if changed:
                bb.instructions = new


def _inject_axon_hooks():
    """Make trace=True work (and a BASS_TRACE env var safe) in this container."""
    if "antenv.axon_hooks" not in sys.modules:
        try:
            from trn_agent_boot.trn_boot import _ntff_profile_via_ctypes

            hook = _ntff_profile_via_ctypes("/opt/axon/libaxon_pjrt.so")
        except Exception:
            hook = None
        mod = types.ModuleType("antenv.axon_hooks")
        mod.get_axon_ntff_profile_hook = lambda: hook
        mod.set_axon_ntff_profile_hook = lambda h: None
        sys.modules["antenv.axon_hooks"] = mod
    from concourse import bass_utils

    bass_utils.upload_artifacts = lambda d: d


def _build(with_b_in, with_b_yo, with_b_head):
    import concourse.bass as bass
    import concourse.mybir as mybir
    from concourse.tile import TileContext
    from concourse.masks import make_identity

    dt = mybir.dt
    AF = mybir.ActivationFunctionType
    OP = mybir.AluOpType

    nc = bass.Bass()
    obsT_e = nc.declare_dram_parameter("obsT", [OBSD, T], dt.bfloat16, isOutput=False)
    w_in_e = nc.declare_dram_parameter("w_in", [OBSD, D], dt.bfloat16, isOutput=False)
    # pre-transposed on the host: one contiguous run per partition per block
    w_dt_e = nc.declare_dram_parameter("w_dt", [P, KT * H], dt.bfloat16, isOutput=False)
    w_b_e = nc.declare_dram_parameter("w_b", [P, MT * KT * P], dt.bfloat16, isOutput=False)
    w_c_e = nc.declare_dram_parameter("w_c", [P, MT * KT * P], dt.bfloat16, isOutput=False)
    w_yo_e = nc.declare_dram_parameter("w_yo", [P, KT * U], dt.bfloat16, isOutput=False)
    w_hd_e = nc.declare_dram_parameter("w_hd", [P, 2 * A], dt.bfloat16, isOutput=False)
    neg_a_e = nc.declare_dram_parameter("neg_a", [H, 1], dt.float32, isOutput=False)
    dtb_e = nc.declare_dram_parameter("dtb", [H, 1], dt.float32, isOutput=False)
    bin_e = byo_e = bhd_e = None
    if with_b_in:
        bin_e = nc.declare_dram_parameter("b_in", [P, KT], dt.float32, isOutput=False)
    if with_b_yo:
        byo_e = nc.declare_dram_parameter("b_yo", [P, 2], dt.float32, isOutput=False)
    if with_b_head:
        bhd_e = nc.declare_dram_parameter("b_hd", [1, A], dt.bfloat16, isOutput=False)
    out_e = nc.declare_dram_parameter("out", [T, A], dt.float32, isOutput=True)

    _patch_tile()
    with TileContext(nc) as tc:
        with (
            tc.tile_pool(name="cst", bufs=1) as cst,
            tc.tile_pool(name="wrk", bufs=2) as wrk,
            tc.tile_pool(name="ps_proj", bufs=2, space="PSUM") as ps_proj,
            tc.tile_pool(name="ps_diff", bufs=2, space="PSUM") as ps_diff,
            tc.tile_pool(name="ps_g", bufs=2, space="PSUM") as ps_g,
            tc.tile_pool(name="ps_sd", bufs=1, space="PSUM") as ps_sd,
        ):
            # ---------------- phase 0: warm-up + first DMAs ----------
            # HAM warm-up: PE activity from ~t0 flips the clock gate to
            # 2.4 GHz while obs/W_in stream in.
            warm = cst.tile([P, T], dt.bfloat16, tag="warm")
            nc.gpsimd.memset(warm[:], 0.5)

            obsT = cst.tile([P, 2 * T], dt.bfloat16, tag="obsT")
            for k in range(2):
                nc.sync.dma_start(
                    out=obsT[:, k * T : (k + 1) * T], in_=obsT_e[k * P : (k + 1) * P, :]
                )
            w_in = cst.tile([P, 2 * D], dt.bfloat16, tag="w_in")
            for k in range(2):
                nc.sync.dma_start(
                    out=w_in[:, k * D : (k + 1) * D], in_=w_in_e[k * P : (k + 1) * P, :]
                )
            w_dt = cst.tile([P, KT * H], dt.bfloat16, tag="w_dt")
            nc.sync.dma_start(out=w_dt[:], in_=w_dt_e[:])
            neg_a = cst.tile([H, 1], dt.float32, tag="neg_a")
            nc.sync.dma_start(out=neg_a[:], in_=neg_a_e[:])
            dtb = cst.tile([H, 1], dt.float32, tag="dtb")
            nc.sync.dma_start(out=dtb[:], in_=dtb_e[:])
            if with_b_in:
                b_in = cst.tile([P, KT], dt.float32, tag="b_in")
                nc.sync.dma_start(out=b_in[:], in_=bin_e[:])

            with tc.tile_pool(name="ps_tiny", bufs=1, space="PSUM") as ps_tiny:
                # single accumulating bank: no PSUM rotation waits, so the
                # warm matmuls stream back-to-back from t~0
                pw = ps_proj.tile([P, T], dt.float32, tag="proj", name="warm")
                for i in range(18):
                    nc.tensor.matmul(
                        pw[:], warm[:, 0:P], warm[:], start=(i == 0), stop=(i == 17)
                    )

                # constants (gpsimd/PE while warm-up runs)
                ident_f = cst.tile([P, P], dt.float32, tag="ident_f")
                make_identity(nc, ident_f[:])
                ident_b = cst.tile([N, N], dt.bfloat16, tag="ident_b")
                make_identity(nc, ident_b[:])
                ident_p = cst.tile([P, P], dt.bfloat16, tag="ident_p")
                make_identity(nc, ident_p[:])
                ones_row = cst.tile([1, T], dt.float32, tag="ones_row")
                nc.gpsimd.memset(ones_row[:], 1.0)
                ones_bf = cst.tile([1, T], dt.bfloat16, tag="ones_bf")
                nc.gpsimd.memset(ones_bf[:], 1.0)
                zero_hl = cst.tile([H, L], dt.float32, tag="zero_hl")
                nc.gpsimd.memset(zero_hl[:], 0.0)

                # ---------------- phase 1: in-proj + dt-proj ------------
                # x^T = relu(W_in^T obs^T) (d,t), 16 d-tiles; the dt
                # projection matmuls ride two slots behind their relu.
                xT = cst.tile([P, KT * T], dt.bfloat16, tag="xT")  # (d, t)
                psd = ps_tiny.tile([H, T], dt.float32, tag="tiny", name="psd")

                def emit_inproj(kt):
                    ps = ps_proj.tile([P, T], dt.float32, tag="proj")
                    for ko in range(2):
                        nc.tensor.matmul(
                            ps[:],
                            w_in[:, ko * D + kt * P : ko * D + (kt + 1) * P],
                            obsT[:, ko * T : (ko + 1) * T],
                            start=(ko == 0),
                            stop=(ko == 1),
                        )
                    if with_b_in:
                        nc.scalar.activation(
                            xT[:, kt * T : (kt + 1) * T], ps[:], AF.Relu,
                            bias=b_in[:, kt : kt + 1],
                        )
                    else:
                        nc.scalar.activation(xT[:, kt * T : (kt + 1) * T], ps[:], AF.Relu)

                def emit_dtproj(kt):
                    nc.tensor.matmul(
                        psd[:],
                        w_dt[:, kt * H : (kt + 1) * H],
                        xT[:, kt * T : (kt + 1) * T],
                        start=(kt == 0),
                        stop=(kt == KT - 1),
                    )

                emit_inproj(0)
                emit_inproj(1)
                for kt in range(2, KT):
                    emit_inproj(kt)
                    emit_dtproj(kt - 2)
                emit_dtproj(KT - 2)
                emit_dtproj(KT - 1)

                # ---------------- dt chain, part A (VE/ACT/DMA only) ----
                # softplus via ln(1+exp(.)) — Softplus shares no ACT table
                # with Exp/Ln here; exp/ln/relu/copy live in one table.
                dtraw = cst.tile([H, T], dt.float32, tag="dtraw")
                dtT = cst.tile([H, T], dt.float32, tag="dtT")
                ldec = cst.tile([H, T], dt.float32, tag="ldec")
                pcumT = cst.tile([H, T], dt.float32, tag="pcumT")
                u_all = cst.tile([H, T], dt.bfloat16, tag="u_all")  # exp(Pcum)
                pcumH = cst.tile([H, T], dt.bfloat16, tag="pcumH")
                pcumL = cst.tile([H, T], dt.bfloat16, tag="pcumL")
                npdH = cst.tile([H, T], dt.bfloat16, tag="npdH")
                npdL = cst.tile([H, T], dt.bfloat16, tag="npdL")

                nc.scalar.activation(dtraw[:], psd[:], AF.Exp, bias=dtb[:])
                nc.vector.tensor_scalar_add(dtraw[:], dtraw[:], 1.0)
                nc.scalar.activation(dtT[:], dtraw[:], AF.Ln)
                nc.vector.tensor_scalar_mul(ldec[:], dtT[:], neg_a[:])
                for c in range(NCH):
                    cb = slice(c * L, (c + 1) * L)
                    nc.vector.tensor_tensor_scan(
                        pcumT[:, cb], ldec[:, cb], zero_hl[:], 0.0,
                        op0=OP.add, op1=OP.add,
                    )
                nc.scalar.activation(u_all[:], pcumT[:], AF.Exp)
                logdt = dtraw  # dtraw is dead after dtT; reuse its slot
                nc.scalar.activation(logdt[:], dtT[:], AF.Ln)
                nc.vector.tensor_sub(logdt[:], logdt[:], pcumT[:])  # now -Pcum+logdt
                nc.vector.tensor_copy(pcumH[:], pcumT[:])
                nc.vector.tensor_sub(pcumL[:], pcumT[:], pcumH[:])
                nc.vector.tensor_copy(npdH[:], logdt[:])
                nc.vector.tensor_sub(npdL[:], logdt[:], npdH[:])

                # per-chunk diff-pack tiles for the batched K=16 E matmul:
                # one matmul builds all 4 heads of a group at once.
                #   lhq[c] (16, 4groups*L): row 4*hi+{0..3} = [1, npdH_h,
                #     1, npdL_h] for head h=4*hg+hi, column block hg (j idx).
                #   rpq[c] (16, H*L): row 4*hi+{0..3} = [pcumH_h, 1,
                #     pcumL_h, 1] gated to column block h (zero elsewhere),
                #     so each head's terms only land in its own E block.
                lhq = [
                    cst.tile([H, NCH * L], dt.bfloat16, tag=f"lhq{c}", name=f"lhq{c}")
                    for c in range(NCH)
                ]
                rpq = [
                    cst.tile([H, H * L], dt.bfloat16, tag=f"rpq{c}", name=f"rpq{c}")
                    for c in range(NCH)
                ]
                for c in range(NCH):
                    nc.gpsimd.memset(lhq[c][:], 1.0)
                    nc.gpsimd.memset(rpq[c][:], 0.0)
                for c in range(NCH):
                    cb = slice(c * L, (c + 1) * L)
                    for hi in range(4):
                        # source rows h=4*hg+hi for hg=0..3 (4 partitions,
                        # stride 4); the dst free dim g matches them
                        def gview(t):
                            return t[hi:H:4, cb]

                        def lview(r):
                            return lhq[c][r : r + 1, :].rearrange(
                                "p (g l) -> p g l", g=NCH
                            )

                        nc.sync.dma_start(out=lview(4 * hi + 1), in_=gview(npdH))
                        nc.sync.dma_start(out=lview(4 * hi + 3), in_=gview(npdL))
                        # rpq: block h = (4*hg+hi)*L -> base hi*L, stride 4L
                        def bview(r):
                            return rpq[c][r : r + 1, :].rearrange(
                                "p (g q l) -> p g q l", g=4, q=4
                            )[:, :, hi, :]

                        nc.sync.dma_start(out=bview(4 * hi + 0), in_=gview(pcumH))
                        nc.sync.dma_start(out=bview(4 * hi + 2), in_=gview(pcumL))
                        ones_v = ones_bf[0:1, 0 : NCH * L].rearrange(
                            "p (g l) -> p g l", g=NCH
                        )
                        nc.sync.dma_start(out=bview(4 * hi + 1), in_=ones_v)
                        nc.sync.dma_start(out=bview(4 * hi + 3), in_=ones_v)

                # ---------------- x (t,d) via PE transposes -------------
                x = cst.tile([P, NCH * D], dt.bfloat16, tag="x")  # (t, d)
                for tt in range(NCH):
                    for dg in range(4):
                        psx = ps_proj.tile([P, 4 * P], dt.bfloat16, tag="proj")
                        for k4 in range(4):
                            kt = dg * 4 + k4
                            nc.tensor.transpose(
                                psx[:, k4 * P : (k4 + 1) * P],
                                xT[:, kt * T + tt * P : kt * T + (tt + 1) * P],
                                ident_p[:],
                            )
                        nc.vector.tensor_copy(
                            x[:, tt * D + dg * 512 : tt * D + (dg + 1) * 512], psx[:]
                        )

                # late weight DMAs (off the critical path)
                w_yo = cst.tile([P, KT * U], dt.bfloat16, tag="w_yo")
                nc.sync.dma_start(out=w_yo[:], in_=w_yo_e[:])
                w_hd = cst.tile([P, 2 * A], dt.bfloat16, tag="w_hd")
                nc.sync.dma_start(out=w_hd[:], in_=w_hd_e[:])
                if with_b_yo:
                    b_yo = cst.tile([P, 2], dt.float32, tag="b_yo")
                    nc.sync.dma_start(out=b_yo[:], in_=byo_e[:])
                if with_b_head:
                    b_hd = cst.tile([1, A], dt.bfloat16, tag="b_hd")
                    nc.sync.dma_start(out=b_hd[:], in_=bhd_e[:])

                # ---------------- scan state/staging tensors ------------
                bm = cst.tile([N, H * T], dt.bfloat16, tag="bm")
                cm = cst.tile([N, H * T], dt.bfloat16, tag="cm")
                cw = cst.tile([N, H * T], dt.bfloat16, tag="cw")
                y = cst.tile([P, KT * T], dt.bfloat16, tag="y")  # (d, t)
                # state snapshots after chunks 0/1/2 (chunk -1 state is zero,
                # the post-chunk-3 state is never read): packed (n, h*p)
                s_ck = [
                    cst.tile([N, H * P], dt.bfloat16, tag=f"sck{c}", name=f"sck{c}")
                    for c in range(NCH - 1)
                ]

                cols = cst.tile([P, NCH * 2 * H], dt.float32, tag="cols")
                plrow = cst.tile([1, NCH * H], dt.float32, tag="plrow")
                ulast = cst.tile([1, NCH * H], dt.float32, tag="ulast")
                dtotc = cst.tile([P, NCH * H], dt.float32, tag="dtotc")
                e2c = cst.tile([P, NCH * H], dt.float32, tag="e2c")
                wcols = cst.tile([P, NCH * H], dt.float32, tag="wcols")

                # ---------------- emitters ------------------------------
                def emit_bc_stage(mt, which):
                    """Issue the weight-block DMA for bc block mt (1 descriptor
                    per partition thanks to the host-side pre-transpose)."""
                    src = w_b_e if which == "b" else w_c_e
                    buf = wrk.tile(
                        [P, KT * P], dt.bfloat16, tag="wstage", bufs=2,
                        name=f"wst_{which}{mt}",
                    )
                    nc.sync.dma_start(
                        out=buf[:], in_=src[:, mt * KT * P : (mt + 1) * KT * P]
                    )
                    return buf

                staged = {}

                btmp_b = {}

                def emit_bc_proj(mt):
                    """Project W_B / W_C columns for heads (2mt, 2mt+1), repack."""
                    he, ho = 2 * mt, 2 * mt + 1
                    for which, dst in (("b", bm), ("c", cm)):
                        buf = staged.pop((mt, which))
                        ps = ps_proj.tile([P, T], dt.float32, tag="proj")
                        for kt in range(KT):
                            nc.tensor.matmul(
                                ps[:],
                                buf[:, kt * P : (kt + 1) * P],
                                xT[:, kt * T : (kt + 1) * T],
                                start=(kt == 0),
                                stop=(kt == KT - 1),
                            )
                        tmp = wrk.tile(
                            [P, T], dt.bfloat16, tag="bctmp", bufs=4,
                            name=f"{which}tmp{mt}",
                        )
                        nc.scalar.activation(tmp[:], ps[:], AF.Copy)
                        if which == "b":
                            # the stacked head-pair layout feeds the paired
                            # btr transposes in emit_scan_sd directly
                            btmp_b[mt] = tmp
                        nc.sync.dma_start(
                            out=dst[:, he * T : (he + 1) * T], in_=tmp[0:N, :]
                        )
                        nc.sync.dma_start(
                            out=dst[:, ho * T : (ho + 1) * T], in_=tmp[N:P, :]
                        )

                def emit_dt_pe1():
                    """cols: transposed (Pcum | dt) columns for all chunks."""
                    pt = ps_tiny.tile([P, 2 * NCH * H], dt.float32, tag="tiny")
                    for c in range(NCH):
                        cb = slice(c * L, (c + 1) * L)
                        nc.tensor.transpose(
                            pt[:, c * 2 * H : c * 2 * H + H],
                            pcumT[:, cb], ident_f[0:H, 0:H],
                        )
                        nc.tensor.transpose(
                            pt[:, c * 2 * H + H : (c + 1) * 2 * H],
                            dtT[:, cb], ident_f[0:H, 0:H],
                        )
                    nc.vector.tensor_copy(cols[:], pt[:])
                    # PcumLast per head at base partition 0 (row 127 of PcumCol)
                    nc.sync.dma_start(
                        out=plrow[:].rearrange("p (c h) -> p c h", c=NCH),
                        in_=cols[L - 1 : L, :].rearrange(
                            "p (c kh) -> p c kh", c=NCH
                        )[:, :, 0:H],
                    )
                    nc.scalar.activation(ulast[:], plrow[:], AF.Exp)

                def emit_dt_pe2():
                    """Broadcast ulast/plast down 128 partitions; derive
                    dtotc / e2c / wcols for all chunks in one go."""
                    pb = ps_tiny.tile([P, 2 * NCH * H], dt.float32, tag="tiny")
                    nc.tensor.matmul(
                        pb[:, 0 : NCH * H], ones_row[0:1, 0:P], ulast[:],
                        start=True, stop=True,
                    )
                    nc.tensor.matmul(
                        pb[:, NCH * H : 2 * NCH * H], ones_row[0:1, 0:P], plrow[:],
                        start=True, stop=True,
                    )
                    nc.vector.tensor_copy(dtotc[:], pb[:, 0 : NCH * H])
                    for c in range(NCH):
                        co = c * 2 * H
                        nc.vector.tensor_sub(
                            e2c[:, c * H : (c + 1) * H],
                            pb[:, NCH * H + c * H : NCH * H + (c + 1) * H],
                            cols[:, co : co + H],
                        )
                    nc.scalar.activation(e2c[:], e2c[:], AF.Exp)
                    for c in range(NCH):
                        co = c * 2 * H
                        nc.vector.tensor_mul(
                            wcols[:, c * H : (c + 1) * H],
                            e2c[:, c * H : (c + 1) * H],
                            cols[:, co + H : co + 2 * H],
                        )

                urows = {}

                def prefetch_urow(mt):
                    """Stage u rows for block mt early, so the K=1 broadcast
                    matmuls never stall behind weight transfers."""
                    if mt > 7 or mt in urows:
                        return
                    he, ho = 2 * mt, 2 * mt + 1
                    urow = wrk.tile(
                        [1, 2 * T], dt.bfloat16, tag="urow", bufs=2, name=f"urow{mt}"
                    )
                    nc.sync.dma_start(out=urow[:, 0:T], in_=u_all[he : he + 1, :])
                    nc.sync.dma_start(out=urow[:, T : 2 * T], in_=u_all[ho : ho + 1, :])
                    urows[mt] = urow

                def emit_cw(mt):
                    """cw = cm * u (broadcast u rows via K=1 matmuls, cast,
                    then scale the repacked cm in SBUF)."""
                    he, ho = 2 * mt, 2 * mt + 1
                    prefetch_urow(mt)
                    urow = urows.pop(mt)
                    prefetch_urow(mt + 1)
                    ubc = wrk.tile([N, 2 * T], dt.bfloat16, tag="ubc", bufs=1)
                    for k in range(2):
                        ubp = ps_proj.tile(
                            [N, T], dt.float32, tag="proj", name=f"ubp{mt}_{k}"
                        )
                        nc.tensor.matmul(
                            ubp[:], ones_bf[0:1, 0:N], urow[:, k * T : (k + 1) * T],
                            start=True, stop=True,
                        )
                        if k == 0:
                            nc.scalar.activation(ubc[:, 0:T], ubp[:], AF.Copy)
                        else:
                            nc.vector.tensor_copy(ubc[:, T : 2 * T], ubp[:])
                    nc.gpsimd.tensor_mul(
                        cw[:, he * T : (he + 1) * T],
                        cm[:, he * T : (he + 1) * T],
                        ubc[:, 0:T],
                    )
                    nc.vector.tensor_mul(
                        cw[:, ho * T : (ho + 1) * T],
                        cm[:, ho * T : (ho + 1) * T],
                        ubc[:, T : 2 * T],
                    )

                yv = y[:].rearrange("p (h t) -> p h t", h=KT)  # (128, 16, 512)
                scan_gw = {}
                zps = [None, None]

                def emit_scan_front(hg, c):
                    """Scan group front: one K=16 E matmul + G matmuls,
                    then the exp/mask/mul chain."""
                    dbank = ps_diff.tile([P, 4 * L], dt.float32, tag="diff")
                    gbank = ps_g.tile([P, 4 * L], dt.float32, tag="g")
                    nc.tensor.matmul(
                        dbank[:],
                        lhq[c][:, hg * L : (hg + 1) * L],
                        rpq[c][:, hg * 4 * L : (hg + 1) * 4 * L],
                        start=True,
                        stop=True,
                    )
                    for hi in range(4):
                        h = hg * 4 + hi
                        hb = slice(h * T + c * L, h * T + (c + 1) * L)
                        nc.tensor.matmul(
                            gbank[:, hi * L : (hi + 1) * L],
                            bm[:, hb],
                            cm[:, hb],
                            start=True,
                            stop=True,
                        )
                    e_sb = wrk.tile([P, 4 * L], dt.bfloat16, tag="e_sb", bufs=3)
                    nc.scalar.activation(e_sb[:], dbank[:], AF.Exp)
                    # causal mask: keep i>=j else 0 (kills the exp-overflow infs)
                    nc.gpsimd.affine_select(
                        out=e_sb[:],
                        in_=e_sb[:],
                        compare_op=OP.is_ge,
                        fill=0.0,
                        base=0,
                        pattern=[[0, 4], [1, L]],
                        channel_multiplier=-1,
                    )
                    gw = wrk.tile([P, 4 * L], dt.bfloat16, tag="gw", bufs=4)
                    nc.vector.tensor_mul(gw[:], gbank[:], e_sb[:])
                    scan_gw[(hg, c)] = gw

                def emit_scan_sd(hg):
                    """State machinery for all 4 chunks of a head group.
                    Depends only on bm/x/dt-chain (NOT on the fronts), so the
                    whole state chain resolves early and the Y matmuls later
                    never wait on a serial VE chain."""
                    for c in range(NCH):
                        # btr shares the diff pool's banks (tag-shared
                        # rotation); the slot's prior dbank is drained by then.
                        btr = ps_diff.tile(
                            [P, 4 * N], dt.bfloat16, tag="diff", name=f"btr{hg}_{c}"
                        )
                        sdb = ps_sd.tile([N, 4 * P], dt.float32, tag="sd")
                        bd = wrk.tile([P, 4 * N], dt.bfloat16, tag="bd")
                        # paired transposes (both heads of a bc block in one
                        # 128-wide op, straight from the un-repacked btmp),
                        # then the sd matmuls: the bd-scale round-trip hides
                        # behind the remaining transposes.
                        cb = slice(c * L, (c + 1) * L)
                        for hi2 in range(2):
                            nc.tensor.transpose(
                                btr[:, hi2 * 2 * N : (hi2 + 1) * 2 * N],
                                btmp_b[2 * hg + hi2][:, cb],
                                ident_p[:],
                            )
                        for hi in range(4):
                            h = hg * 4 + hi
                            # bd = btr * (e2c*dt): split evacs ACT/DVE
                            wc = wcols[:, c * H + h : c * H + h + 1]
                            if hi % 2 == 0:
                                nc.scalar.activation(
                                    bd[:, hi * N : (hi + 1) * N],
                                    btr[:, hi * N : (hi + 1) * N],
                                    AF.Copy,
                                    scale=wc,
                                )
                            else:
                                nc.vector.tensor_scalar_mul(
                                    bd[:, hi * N : (hi + 1) * N],
                                    btr[:, hi * N : (hi + 1) * N],
                                    wc,
                                )
                        for hi in range(4):
                            h = hg * 4 + hi
                            xc = x[:, c * D + h * P : c * D + (h + 1) * P]
                            nc.tensor.matmul(
                                sdb[:, hi * P : (hi + 1) * P],
                                bd[:, hi * N : (hi + 1) * N], xc,
                                start=True, stop=True,
                            )
                        gs = slice(hg * 4 * P, (hg + 1) * 4 * P)
                        if c == 0:
                            # S_after_0 = sds_0 (prior state is zero)
                            nc.vector.tensor_copy(s_ck[0][0:N, gs], sdb[:])
                        elif c < NCH - 1:
                            for hi in range(4):
                                h = hg * 4 + hi
                                nc.vector.scalar_tensor_tensor(
                                    s_ck[c][0:N, h * P : (h + 1) * P],
                                    s_ck[c - 1][0:N, h * P : (h + 1) * P],
                                    dtotc[0:N, c * H + h : c * H + h + 1],
                                    sdb[:, hi * P : (hi + 1) * P],
                                    op0=OP.mult,
                                    op1=OP.add,
                                )
                        # c == NCH-1: the post-chunk-3 state is never read

                def emit_scan_y(hg, c, ps_y):
                    """Y^T = x_chunk^T Gw (+ S_prev^T cw for c>0), evac to y."""
                    gw = scan_gw.pop((hg, c))
                    cb = slice(c * L, (c + 1) * L)
                    ybank = ps_y.tile([P, 4 * L], dt.float32, tag="y")
                    for hi in range(4):
                        h = hg * 4 + hi
                        hb = slice(h * T + c * L, h * T + (c + 1) * L)
                        xc = x[:, c * D + h * P : c * D + (h + 1) * P]
                        nc.tensor.matmul(
                            ybank[:, hi * L : (hi + 1) * L],
                            xc,
                            gw[:, hi * L : (hi + 1) * L],
                            start=True,
                            stop=(c == 0),
                        )
                        if c > 0:
                            nc.tensor.matmul(
                                ybank[:, hi * L : (hi + 1) * L],
                                s_ck[c - 1][0:N, h * P : (h + 1) * P],
                                cw[:, hb],
                                start=False,
                                stop=True,
                            )
                    # Y evac: psum (p, 4*L) -> y (d,t) blocks [h, c*L:(c+1)*L]
                    nc.scalar.activation(
                        yv[:, hg * 4 : hg * 4 + 4, cb],
                        ybank[:].rearrange("p (h t) -> p h t", h=4),
                        AF.Copy,
                    )

                def emit_z_alloc():
                    zps[0] = ps_proj.tile([P, T], dt.float32, tag="proj", name="zps0")
                    zps[1] = ps_proj.tile([P, T], dt.float32, tag="proj", name="zps1")

                def emit_z(h):
                    """Accumulate head h's slice of z = W_yo^T y."""
                    for ut in range(2):
                        nc.tensor.matmul(
                            zps[ut][:],
                            w_yo[:, h * U + ut * P : h * U + (ut + 1) * P],
                            y[:, h * T : (h + 1) * T],
                            start=(h == 0),
                            stop=(h == H - 1),
                        )

                # ---------------- phase 2 schedule ----------------------
                # PE backbone: bc blocks + z; the state machinery (SD),
                # fronts (F) and Y matmuls slot between them as soon as
                # their deps land. All serial chains resolve early.
                staged[(0, "b")] = emit_bc_stage(0, "b")
                staged[(0, "c")] = emit_bc_stage(0, "c")
                staged[(1, "b")] = emit_bc_stage(1, "b")
                staged[(1, "c")] = emit_bc_stage(1, "c")
                emit_bc_proj(0)
                emit_dt_pe1()
                staged[(2, "b")] = emit_bc_stage(2, "b")
                staged[(2, "c")] = emit_bc_stage(2, "c")
                emit_bc_proj(1)
                emit_dt_pe2()
                prefetch_urow(0)

            emit_cw(0)
            emit_cw(1)
            emit_scan_sd(0)
            emit_scan_front(0, 0)
            emit_scan_front(0, 1)
            staged[(3, "b")] = emit_bc_stage(3, "b")
            staged[(3, "c")] = emit_bc_stage(3, "c")
            emit_bc_proj(2)
            emit_scan_front(0, 2)
            emit_scan_front(0, 3)
            with tc.tile_pool(name="ps_y", bufs=1, space="PSUM") as ps_y:
                emit_scan_y(0, 0, ps_y)
                emit_scan_y(0, 1, ps_y)
                staged[(4, "b")] = emit_bc_stage(4, "b")
                staged[(4, "c")] = emit_bc_stage(4, "c")
                emit_bc_proj(3)
                emit_scan_y(0, 2, ps_y)
                emit_scan_y(0, 3, ps_y)
                emit_scan_sd(1)
                emit_cw(2)
                emit_cw(3)
                emit_scan_front(1, 0)
                emit_scan_front(1, 1)
                staged[(5, "b")] = emit_bc_stage(5, "b")
                staged[(5, "c")] = emit_bc_stage(5, "c")
                emit_bc_proj(4)
                emit_scan_front(1, 2)
                emit_scan_front(1, 3)
                emit_scan_y(1, 0, ps_y)
                emit_scan_y(1, 1, ps_y)
                staged[(6, "b")] = emit_bc_stage(6, "b")
                staged[(6, "c")] = emit_bc_stage(6, "c")
                emit_bc_proj(5)
                emit_scan_y(1, 2, ps_y)
                emit_scan_y(1, 3, ps_y)
                emit_scan_sd(2)
                emit_cw(4)
                emit_cw(5)
                emit_scan_front(2, 0)
                emit_scan_front(2, 1)
                staged[(7, "b")] = emit_bc_stage(7, "b")
                staged[(7, "c")] = emit_bc_stage(7, "c")
                emit_bc_proj(6)
                emit_scan_front(2, 2)
                emit_scan_front(2, 3)
                emit_scan_y(2, 0, ps_y)
                emit_scan_y(2, 1, ps_y)
                emit_cw(6)
                emit_bc_proj(7)
                emit_scan_y(2, 2, ps_y)
                emit_scan_y(2, 3, ps_y)
                emit_scan_sd(3)
                emit_cw(7)
                emit_scan_front(3, 0)
                emit_scan_front(3, 1)
                emit_scan_front(3, 2)
                emit_scan_front(3, 3)
                emit_z_alloc()
                emit_z(0)
                emit_z(1)
                emit_z(2)
                emit_z(3)
                emit_scan_y(3, 0, ps_y)
                emit_z(4)
                emit_z(5)
                emit_scan_y(3, 1, ps_y)
                emit_z(6)
                emit_z(7)
                emit_scan_y(3, 2, ps_y)
                emit_z(8)
                emit_z(9)
                emit_z(10)
                emit_z(11)
                emit_scan_y(3, 3, ps_y)
                emit_z(12)
                emit_z(13)
                emit_z(14)
                emit_z(15)

                # ---------------- tail: zT + logits ---------------------
                zT = cst.tile([P, 2 * T], dt.bfloat16, tag="zT")  # (u, t)
                for ut in range(2):
                    if with_b_yo:
                        nc.scalar.activation(
                            zT[:, ut * T : (ut + 1) * T], zps[ut][:], AF.Relu,
                            bias=b_yo[:, ut : ut + 1],
                        )
                    else:
                        nc.scalar.activation(
                            zT[:, ut * T : (ut + 1) * T], zps[ut][:], AF.Relu
                        )

                logit = cst.tile([P, NCH * A], dt.float32, tag="logit")
                for tt in range(NCH):
                    ps = ps_y.tile([P, A], dt.float32, tag="y", name=f"lg{tt}")
                    nmm = 3 if with_b_head else 2
                    for ut in range(2):
                        nc.tensor.matmul(
                            ps[:],
                            zT[:, ut * T + tt * P : ut * T + (tt + 1) * P],
                            w_hd[:, ut * A : (ut + 1) * A],
                            start=(ut == 0),
                            stop=(ut == nmm - 1),
                        )
                    if with_b_head:
                        nc.tensor.matmul(
                            ps[:],
                            ones_bf[0:1, tt * P : (tt + 1) * P],
                            b_hd[:],
                            start=False,
                            stop=True,
                        )
                    nc.scalar.activation(logit[:, tt * A : (tt + 1) * A], ps[:], AF.Copy)
                    nc.sync.dma_start(
                        out=out_e[tt * P : (tt + 1) * P, :],
                        in_=logit[:, tt * A : (tt + 1) * A],
                    )

    _split_multi_waits(nc)
    return nc


def kernel(obs, W_in, b_in, A_log, dt_bias, W_dt, W_B, W_C, W_yo, b_yo, W_head, b_head):
    _inject_axon_hooks()
    _patch_tile()
    from concourse.bass_utils import run_bass_kernel_spmd

    obs = np.asarray(obs, dtype=np.float32)
    flags = (
        bool(np.any(np.asarray(b_in) != 0)),
        bool(np.any(np.asarray(b_yo) != 0)),
        bool(np.any(np.asarray(b_head) != 0)),
    )
    # First call: build once (the verified path). Repeat calls in one
    # process rebuild a fresh graph — re-executing a previously-run nc with
    # new inputs has crashed the exec unit (NRT status 101) in testing.
    if flags not in _CACHE:
        _CACHE[flags] = _build(*flags)
    elif _EXECUTED.get(flags):
        _CACHE[flags] = _build(*flags)
    nc = _CACHE[flags]
    _EXECUTED[flags] = True

    obsT = obs.reshape(T, BSZ, OBSD).transpose(1, 2, 0)  # (B, 256, T)

    def colblocks(w, blk):
        # (D, M) -> (P, M//blk * KT * blk): per-partition contiguous blocks
        m = w.shape[1]
        return np.ascontiguousarray(
            w.reshape(KT, P, m // blk, blk).transpose(1, 2, 0, 3).reshape(P, -1)
        )

    base = {
        "w_in": np.ascontiguousarray(W_in).astype(BF16),
        "w_dt": colblocks(np.asarray(W_dt), H).astype(BF16),
        "w_b": colblocks(np.asarray(W_B), P).astype(BF16),
        "w_c": colblocks(np.asarray(W_C), P).astype(BF16),
        "w_yo": colblocks(np.asarray(W_yo), U).astype(BF16),
        "w_hd": np.ascontiguousarray(
            np.asarray(W_head).reshape(2, P, A).transpose(1, 0, 2).reshape(P, 2 * A)
        ).astype(BF16),
        "neg_a": (-np.exp(np.asarray(A_log, np.float64)))
        .astype(np.float32)
        .reshape(H, 1),
        "dtb": np.asarray(dt_bias, np.float32).reshape(H, 1),
    }
    if flags[0]:
        base["b_in"] = np.ascontiguousarray(
            np.asarray(b_in, np.float32).reshape(KT, P).T
        )
    if flags[1]:
        base["b_yo"] = np.ascontiguousarray(
            np.asarray(b_yo, np.float32).reshape(2, P).T
        )
    if flags[2]:
        base["b_hd"] = np.asarray(b_head).astype(BF16).reshape(1, A)
    in_maps = [
        dict(base, obsT=np.ascontiguousarray(obsT[c]).astype(BF16)) for c in range(BSZ)
    ]
    global _last_in_maps
    _last_in_maps = in_maps
    res = run_bass_kernel_spmd(nc, in_maps, core_ids=list(range(BSZ)))
    out = np.stack([res.results[c]["out"] for c in range(BSZ)], axis=1)
    return out.astype(np.float32)


# revision 24
# speedup vs baseline: 1.0618x; 1.0618x over previous
"""Self-contained Trainium2 kernel for the SSD-scan actor network.

Data-parallel over batch B=8 across 8 NeuronCores (one sample per core, no
collectives). Per core:
  x  = relu(obs @ W_in + b_in)                  (T=512, D=2048)
  dt = softplus(x @ W_dt + dt_bias)             (T, H=16)
  Bm = x @ W_B, Cm = x @ W_C                    (T, H, N=64)
  y  = selective scan over T (Mamba2 SSD)       (T, D)
  z  = relu(y @ W_yo + b_yo)                    (T, U=256)
  out = z @ W_head + b_head                     (T, A=64)

The scan uses the chunked (segsum) SSD formulation: chunk length L=128,
4 chunks, 16 independent heads. Per head/chunk:
  E[j,i]  = exp(Pcum_i - Pcum_j + log dt_j), causally masked to j<=i
  Y^T     = x_chunk^T Gw + S_prev^T (C*u),  Gw = (B C^T)^T . E, u_i=exp(Pcum_i)
  S_new   = exp(Pcum_L-1) S_prev + sum_j exp(Pcum_L-1 - Pcum_j) dt_j B_j x_j^T
Big matmuls run in bf16 with fp32 PSUM accumulation; the Diff matrix
(Pcum_i - Pcum_j + logdt_j) is built exactly with K=4 bf16 (hi/lo) matmuls.

v8 schedule: the PE instruction stream is kept dense end-to-end so the HAM
activity monitor never demotes the clock to 1.2 GHz:
  - weights are pre-transposed on the host so every SBUF load is one
    contiguous descriptor per partition (the old strided loads saturated
    the DMA/sync engines);
  - the dt chain is batched across all 4 chunks (cumsum via the DVE
    tensor_tensor_scan op) so its PE footprint is ~10 small ops that are
    slotted between the B/C projection blocks instead of serializing them;
  - scan groups start as soon as their B/C block lands; the z projection
    and the last head-group's chunk chain share the tail.

Hardware notes (all discovered the hard way on this container's stack):
  - walrus here allows only ONE sync wait per instruction -> _split_multi_waits
  - matmul operands whose APs start at partition 64 crash the exec unit
    (NRT_EXEC_UNIT_UNRECOVERABLE), so every matmul operand is kept at base
    partition 0: B/C are repacked to 64-partition tensors via bf16
    staging + SBUF-to-SBUF DMA (DMA moves across partitions; DVE cannot).
  - Softplus shares no ACT function table with Exp/Ln -> ln(1+exp(x)).
"""

import sys
import types

import numpy as np
import ml_dtypes

T, BSZ, OBSD = 512, 8, 256
D, H, N, P = 2048, 16, 64, 128
U, A = 256, 64
L, NCH, KT = 128, 4, 16  # chunk length, #chunks, #d-tiles (D/128)
MT = 8  # B/C column blocks (HN/128)
BF16 = ml_dtypes.bfloat16

_CACHE = {}
_EXECUTED = {}


def _patch_tile():
    """Split the TileContext final drain's waits across single-wait nops."""
    from concourse import tile, mybir
    from concourse.vector_clock import ScopedClock

    if getattr(tile.TileContext, "_drain_patched", False):
        return

    def _patched(self, tick_clock, wait_clock):
        nc = self.nc
        probe = nc.sync.nop()
        wait_clock.add_sem_waits(
            probe.ins, ScopedClock({None: tick_clock.global_clock})
        )
        si = probe.ins.sync_info
        if si is not None and len(si.on_wait) > 1:
            waits = list(si.on_wait)
            probe.ins.sync_info = mybir.SyncInfo(
                on_wait=[waits[0]], on_update=list(si.on_update)
            )
            for w in waits[1:]:
                nop = nc.sync.nop()
                nop.ins.sync_info = mybir.SyncInfo(on_wait=[w], on_update=[])
        nc.sync.drain()
        nc.all_engine_barrier(sem_only=True)
        assert self.sems is not None
        popped = nc._tile_sem_poison_stack.pop()
        assert popped is self._sem_poison
        nc.clear_and_free_semaphores(list(self.sems.allocated().values()))
        # NOTE: the stock drain runs a second all_engine_barrier here (~5us
        # of ring latency); nothing uses the cleared semaphores afterwards —
        # the NEFF ends — so it is skipped.

    tile.TileContext._drain_and_barrier = _patched
    tile.TileContext._drain_patched = True


def _split_multi_waits(nc):
    """This walrus build accepts at most one sync wait per instruction.
    Hoist extra waits onto single-wait NoOps inserted just before, on the
    same engine (the sequencer stalls there first — strictly conservative)."""
    from concourse import mybir

    n = 0
    for f in nc.m.functions:
        for bb in f.blocks:
            insts = list(bb.instructions)
            changed = False
            new = []
            for inst in insts:
                try:
                    si = inst.sync_info
                except Exception:
                    si = None
                if si is not None and len(si.on_wait) > 1:
                    waits = list(si.on_wait)
                    for w in waits[:-1]:
                        nop = mybir.InstNoOp(
                            name=f"wsplit-{n}", ins=[], outs=[], engine=inst.engine
                        )
                        n += 1
                        nop.sync_info = mybir.SyncInfo(on_wait=[w], on_update=[])
                        nc.register_instruction(nop, overwrite=True)
                        new.append(nop)
                    inst.sync_info = mybir.SyncInfo(
                        on_wait=[waits[-1]], on_update=list(si.on_update)
                    )
                    changed = True
                new.append(inst)
            if changed:
                bb.instructions = new


def _inject_axon_hooks():
    """Make trace=True work (and a BASS_TRACE env var safe) in this container."""
    if "antenv.axon_hooks" not in sys.modules:
        try:
            from trn_agent_boot.trn_boot import _ntff_profile_via_ctypes

            hook = _ntff_profile_via_ctypes("/opt/axon/libaxon_pjrt.so")
        except Exception:
            hook = None
        mod = types.ModuleType("antenv.axon_hooks")
        mod.get_axon_ntff_profile_hook = lambda: hook
        mod.set_axon_ntff_profile_hook = lambda h: None
        sys.modules["antenv.axon_hooks"] = mod
    from concourse import bass_utils

    bass_utils.upload_artifacts = lambda d: d


def _build(with_b_in, with_b_yo, with_b_head):
    import concourse.bass as bass
    import concourse.mybir as mybir
    from concourse.tile import TileContext
    from concourse.masks import make_identity

    dt = mybir.dt
    AF = mybir.ActivationFunctionType
    OP = mybir.AluOpType

    nc = bass.Bass()
    obsT_e = nc.declare_dram_parameter("obsT", [OBSD, T], dt.bfloat16, isOutput=False)
    w_in_e = nc.declare_dram_parameter("w_in", [OBSD, D], dt.bfloat16, isOutput=False)
    # pre-transposed on the host: one contiguous run per partition per block
    w_dt_e = nc.declare_dram_parameter("w_dt", [P, KT * H], dt.bfloat16, isOutput=False)
    w_b_e = nc.declare_dram_parameter("w_b", [P, MT * KT * P], dt.bfloat16, isOutput=False)
    w_c_e = nc.declare_dram_parameter("w_c", [P, MT * KT * P], dt.bfloat16, isOutput=False)
    w_yo_e = nc.declare_dram_parameter("w_yo", [P, KT * U], dt.bfloat16, isOutput=False)
    w_hd_e = nc.declare_dram_parameter("w_hd", [P, 2 * A], dt.bfloat16, isOutput=False)
    neg_a_e = nc.declare_dram_parameter("neg_a", [H, 1], dt.float32, isOutput=False)
    dtb_e = nc.declare_dram_parameter("dtb", [H, 1], dt.float32, isOutput=False)
    bin_e = byo_e = bhd_e = None
    if with_b_in:
        bin_e = nc.declare_dram_parameter("b_in", [P, KT], dt.float32, isOutput=False)
    if with_b_yo:
        byo_e = nc.declare_dram_parameter("b_yo", [P, 2], dt.float32, isOutput=False)
    if with_b_head:
        bhd_e = nc.declare_dram_parameter("b_hd", [1, A], dt.bfloat16, isOutput=False)
    out_e = nc.declare_dram_parameter("out", [T, A], dt.float32, isOutput=True)

    _patch_tile()
    with TileContext(nc) as tc:
        with (
            tc.tile_pool(name="cst", bufs=1) as cst,
            tc.tile_pool(name="wrk", bufs=2) as wrk,
            tc.tile_pool(name="ps_proj", bufs=2, space="PSUM") as ps_proj,
            tc.tile_pool(name="ps_diff", bufs=2, space="PSUM") as ps_diff,
            tc.tile_pool(name="ps_g", bufs=2, space="PSUM") as ps_g,
            tc.tile_pool(name="ps_sd", bufs=1, space="PSUM") as ps_sd,
        ):
            # ---------------- phase 0: warm-up + first DMAs ----------
            # HAM warm-up: PE activity from ~t0 flips the clock gate to
            # 2.4 GHz while obs/W_in stream in.
            warm = cst.tile([P, T], dt.bfloat16, tag="warm")
            nc.gpsimd.memset(warm[:], 0.5)

            obsT = cst.tile([P, 2 * T], dt.bfloat16, tag="obsT")
            for k in range(2):
                nc.sync.dma_start(
                    out=obsT[:, k * T : (k + 1) * T], in_=obsT_e[k * P : (k + 1) * P, :]
                )
            w_in = cst.tile([P, 2 * D], dt.bfloat16, tag="w_in")
            for k in range(2):
                nc.sync.dma_start(
                    out=w_in[:, k * D : (k + 1) * D], in_=w_in_e[k * P : (k + 1) * P, :]
                )
            w_dt = cst.tile([P, KT * H], dt.bfloat16, tag="w_dt")
            nc.sync.dma_start(out=w_dt[:], in_=w_dt_e[:])
            neg_a = cst.tile([H, 1], dt.float32, tag="neg_a")
            nc.sync.dma_start(out=neg_a[:], in_=neg_a_e[:])
            dtb = cst.tile([H, 1], dt.float32, tag="dtb")
            nc.sync.dma_start(out=dtb[:], in_=dtb_e[:])
            if with_b_in:
                b_in = cst.tile([P, KT], dt.float32, tag="b_in")
                nc.sync.dma_start(out=b_in[:], in_=bin_e[:])

            with tc.tile_pool(name="ps_tiny", bufs=1, space="PSUM") as ps_tiny:
                # single accumulating bank: no PSUM rotation waits, so the
                # warm matmuls stream back-to-back from t~0
                pw = ps_proj.tile([P, T], dt.float32, tag="proj", name="warm")
                for i in range(18):
                    nc.tensor.matmul(
                        pw[:], warm[:, 0:P], warm[:], start=(i == 0), stop=(i == 17)
                    )

                # constants (gpsimd/PE while warm-up runs)
                ident_f = cst.tile([P, P], dt.float32, tag="ident_f")
                make_identity(nc, ident_f[:])
                ident_b = cst.tile([N, N], dt.bfloat16, tag="ident_b")
                make_identity(nc, ident_b[:])
                ident_p = cst.tile([P, P], dt.bfloat16, tag="ident_p")
                make_identity(nc, ident_p[:])
                ones_row = cst.tile([1, T], dt.float32, tag="ones_row")
                nc.gpsimd.memset(ones_row[:], 1.0)
                ones_bf = cst.tile([1, T], dt.bfloat16, tag="ones_bf")
                nc.gpsimd.memset(ones_bf[:], 1.0)
                zero_hl = cst.tile([H, L], dt.float32, tag="zero_hl")
                nc.gpsimd.memset(zero_hl[:], 0.0)

                # ---------------- phase 1: in-proj + dt-proj ------------
                # x^T = relu(W_in^T obs^T) (d,t), 16 d-tiles; the dt
                # projection matmuls ride two slots behind their relu.
                xT = cst.tile([P, KT * T], dt.bfloat16, tag="xT")  # (d, t)
                psd = ps_tiny.tile([H, T], dt.float32, tag="tiny", name="psd")

                def emit_inproj(kt):
                    ps = ps_proj.tile([P, T], dt.float32, tag="proj")
                    for ko in range(2):
                        nc.tensor.matmul(
                            ps[:],
                            w_in[:, ko * D + kt * P : ko * D + (kt + 1) * P],
                            obsT[:, ko * T : (ko + 1) * T],
                            start=(ko == 0),
                            stop=(ko == 1),
                        )
                    if with_b_in:
                        nc.scalar.activation(
                            xT[:, kt * T : (kt + 1) * T], ps[:], AF.Relu,
                            bias=b_in[:, kt : kt + 1],
                        )
                    else:
                        nc.scalar.activation(xT[:, kt * T : (kt + 1) * T], ps[:], AF.Relu)

                def emit_dtproj(kt):
                    nc.tensor.matmul(
                        psd[:],
                        w_dt[:, kt * H : (kt + 1) * H],
                        xT[:, kt * T : (kt + 1) * T],
                        start=(kt == 0),
                        stop=(kt == KT - 1),
                    )

                emit_inproj(0)
                emit_inproj(1)
                for kt in range(2, KT):
                    emit_inproj(kt)
                    emit_dtproj(kt - 2)
                emit_dtproj(KT - 2)
                emit_dtproj(KT - 1)

                # ---------------- dt chain, part A (VE/ACT/DMA only) ----
                # softplus via ln(1+exp(.)) — Softplus shares no ACT table
                # with Exp/Ln here; exp/ln/relu/copy live in one table.
                dtraw = cst.tile([H, T], dt.float32, tag="dtraw")
                dtT = cst.tile([H, T], dt.float32, tag="dtT")
                ldec = cst.tile([H, T], dt.float32, tag="ldec")
                pcumT = cst.tile([H, T], dt.float32, tag="pcumT")
                u_all = cst.tile([H, T], dt.bfloat16, tag="u_all")  # exp(Pcum)
                pcumH = cst.tile([H, T], dt.bfloat16, tag="pcumH")
                pcumL = cst.tile([H, T], dt.bfloat16, tag="pcumL")
                npdH = cst.tile([H, T], dt.bfloat16, tag="npdH")
                npdL = cst.tile([H, T], dt.bfloat16, tag="npdL")

                nc.scalar.activation(dtraw[:], psd[:], AF.Exp, bias=dtb[:])
                nc.vector.tensor_scalar_add(dtraw[:], dtraw[:], 1.0)
                nc.scalar.activation(dtT[:], dtraw[:], AF.Ln)
                nc.vector.tensor_scalar_mul(ldec[:], dtT[:], neg_a[:])
                for c in range(NCH):
                    cb = slice(c * L, (c + 1) * L)
                    nc.vector.tensor_tensor_scan(
                        pcumT[:, cb], ldec[:, cb], zero_hl[:], 0.0,
                        op0=OP.add, op1=OP.add,
                    )
                nc.scalar.activation(u_all[:], pcumT[:], AF.Exp)
                logdt = dtraw  # dtraw is dead after dtT; reuse its slot
                nc.scalar.activation(logdt[:], dtT[:], AF.Ln)
                nc.vector.tensor_sub(logdt[:], logdt[:], pcumT[:])  # now -Pcum+logdt
                nc.vector.tensor_copy(pcumH[:], pcumT[:])
                nc.vector.tensor_sub(pcumL[:], pcumT[:], pcumH[:])
                nc.vector.tensor_copy(npdH[:], logdt[:])
                nc.vector.tensor_sub(npdL[:], logdt[:], npdH[:])

                # per-chunk diff-pack tiles for the batched K=16 E matmul:
                # one matmul builds all 4 heads of a group at once.
                #   lhq[c] (16, 4groups*L): row 4*hi+{0..3} = [1, npdH_h,
                #     1, npdL_h] for head h=4*hg+hi, column block hg (j idx).
                #   rpq[c] (16, H*L): row 4*hi+{0..3} = [pcumH_h, 1,
                #     pcumL_h, 1] gated to column block h (zero elsewhere),
                #     so each head's terms only land in its own E block.
                lhq = [
                    cst.tile([H, NCH * L], dt.bfloat16, tag=f"lhq{c}", name=f"lhq{c}")
                    for c in range(NCH)
                ]
                rpq = [
                    cst.tile([H, H * L], dt.bfloat16, tag=f"rpq{c}", name=f"rpq{c}")
                    for c in range(NCH)
                ]
                for c in range(NCH):
                    nc.vector.memset(lhq[c][:], 1.0)
                    nc.vector.memset(rpq[c][:], 0.0)

                def emit_packs(c):
                    # NOTE: these DMAs wait on the dt-chain VE results; they
                    # must sit BEHIND the weight-stage DMAs in the sync
                    # queue or they head-of-line block the bc pipeline.
                    cb = slice(c * L, (c + 1) * L)
                    for hi in range(4):
                        # source rows h=4*hg+hi for hg=0..3 (4 partitions,
                        # stride 4); the dst free dim g matches them
                        def gview(t):
                            return t[hi:H:4, cb]

                        def lview(r):
                            return lhq[c][r : r + 1, :].rearrange(
                                "p (g l) -> p g l", g=NCH
                            )

                        nc.sync.dma_start(out=lview(4 * hi + 1), in_=gview(npdH))
                        nc.sync.dma_start(out=lview(4 * hi + 3), in_=gview(npdL))
                        # rpq: block h = (4*hg+hi)*L -> base hi*L, stride 4L
                        def bview(r):
                            return rpq[c][r : r + 1, :].rearrange(
                                "p (g q l) -> p g q l", g=4, q=4
                            )[:, :, hi, :]

                        nc.sync.dma_start(out=bview(4 * hi + 0), in_=gview(pcumH))
                        nc.sync.dma_start(out=bview(4 * hi + 2), in_=gview(pcumL))
                        ones_v = ones_bf[0:1, 0 : NCH * L].rearrange(
                            "p (g l) -> p g l", g=NCH
                        )
                        nc.sync.dma_start(out=bview(4 * hi + 1), in_=ones_v)
                        nc.sync.dma_start(out=bview(4 * hi + 3), in_=ones_v)

                # ---------------- x (t,d) via PE transposes -------------
                x = cst.tile([P, NCH * D], dt.bfloat16, tag="x")  # (t, d)
                for tt in range(NCH):
                    for dg in range(4):
                        psx = ps_proj.tile([P, 4 * P], dt.bfloat16, tag="proj")
                        for k4 in range(4):
                            kt = dg * 4 + k4
                            nc.tensor.transpose(
                                psx[:, k4 * P : (k4 + 1) * P],
                                xT[:, kt * T + tt * P : kt * T + (tt + 1) * P],
                                ident_p[:],
                            )
                        nc.vector.tensor_copy(
                            x[:, tt * D + dg * 512 : tt * D + (dg + 1) * 512], psx[:]
                        )

                # late weight DMAs (off the critical path)
                w_yo = cst.tile([P, KT * U], dt.bfloat16, tag="w_yo")
                nc.sync.dma_start(out=w_yo[:], in_=w_yo_e[:])
                w_hd = cst.tile([P, 2 * A], dt.bfloat16, tag="w_hd")
                nc.sync.dma_start(out=w_hd[:], in_=w_hd_e[:])
                if with_b_yo:
                    b_yo = cst.tile([P, 2], dt.float32, tag="b_yo")
                    nc.sync.dma_start(out=b_yo[:], in_=byo_e[:])
                if with_b_head:
                    b_hd = cst.tile([1, A], dt.bfloat16, tag="b_hd")
                    nc.sync.dma_start(out=b_hd[:], in_=bhd_e[:])

                # ---------------- scan state/staging tensors ------------
                bm = cst.tile([N, H * T], dt.bfloat16, tag="bm")
                cm = cst.tile([N, H * T], dt.bfloat16, tag="cm")
                cw = cst.tile([N, H * T], dt.bfloat16, tag="cw")
                y = cst.tile([P, KT * T], dt.bfloat16, tag="y")  # (d, t)
                # state snapshots after chunks 0/1/2 (chunk -1 state is zero,
                # the post-chunk-3 state is never read): packed (n, h*p)
                s_ck = [
                    cst.tile([N, H * P], dt.bfloat16, tag=f"sck{c}", name=f"sck{c}")
                    for c in range(NCH - 1)
                ]

                cols = cst.tile([P, NCH * 2 * H], dt.float32, tag="cols")
                plrow = cst.tile([1, NCH * H], dt.float32, tag="plrow")
                ulast = cst.tile([1, NCH * H], dt.float32, tag="ulast")
                dtotc = cst.tile([P, NCH * H], dt.float32, tag="dtotc")
                e2c = cst.tile([P, NCH * H], dt.float32, tag="e2c")
                wcols = cst.tile([P, NCH * H], dt.float32, tag="wcols")

                # ---------------- emitters ------------------------------
                def emit_bc_stage(mt, which):
                    """Issue the weight-block DMA for bc block mt (1 descriptor
                    per partition thanks to the host-side pre-transpose)."""
                    src = w_b_e if which == "b" else w_c_e
                    buf = wrk.tile(
                        [P, KT * P], dt.bfloat16, tag="wstage", bufs=3,
                        name=f"wst_{which}{mt}",
                    )
                    nc.sync.dma_start(
                        out=buf[:], in_=src[:, mt * KT * P : (mt + 1) * KT * P]
                    )
                    return buf

                staged = {}

                btmp_b = {}

                def emit_bc_proj(mt):
                    """Project W_B / W_C columns for heads (2mt, 2mt+1), repack."""
                    he, ho = 2 * mt, 2 * mt + 1
                    for which, dst in (("b", bm), ("c", cm)):
                        buf = staged.pop((mt, which))
                        ps = ps_proj.tile([P, T], dt.float32, tag="proj")
                        for kt in range(KT):
                            nc.tensor.matmul(
                                ps[:],
                                buf[:, kt * P : (kt + 1) * P],
                                xT[:, kt * T : (kt + 1) * T],
                                start=(kt == 0),
                                stop=(kt == KT - 1),
                            )
                        tmp = wrk.tile(
                            [P, T], dt.bfloat16, tag="bctmp", bufs=4,
                            name=f"{which}tmp{mt}",
                        )
                        nc.scalar.activation(tmp[:], ps[:], AF.Copy)
                        if which == "b":
                            # the stacked head-pair layout feeds the paired
                            # btr transposes in emit_scan_sd directly
                            btmp_b[mt] = tmp
                        nc.sync.dma_start(
                            out=dst[:, he * T : (he + 1) * T], in_=tmp[0:N, :]
                        )
                        nc.sync.dma_start(
                            out=dst[:, ho * T : (ho + 1) * T], in_=tmp[N:P, :]
                        )

                def emit_dt_pe1():
                    """cols: transposed (Pcum | dt) columns for all chunks."""
                    pt = ps_tiny.tile([P, 2 * NCH * H], dt.float32, tag="tiny")
                    for c in range(NCH):
                        cb = slice(c * L, (c + 1) * L)
                        nc.tensor.transpose(
                            pt[:, c * 2 * H : c * 2 * H + H],
                            pcumT[:, cb], ident_f[0:H, 0:H],
                        )
                        nc.tensor.transpose(
                            pt[:, c * 2 * H + H : (c + 1) * 2 * H],
                            dtT[:, cb], ident_f[0:H, 0:H],
                        )
                    nc.vector.tensor_copy(cols[:], pt[:])
                    # PcumLast per head at base partition 0 (row 127 of PcumCol)
                    nc.sync.dma_start(
                        out=plrow[:].rearrange("p (c h) -> p c h", c=NCH),
                        in_=cols[L - 1 : L, :].rearrange(
                            "p (c kh) -> p c kh", c=NCH
                        )[:, :, 0:H],
                    )
                    nc.scalar.activation(ulast[:], plrow[:], AF.Exp)

                def emit_dt_pe2():
                    """Broadcast ulast/plast down 128 partitions; derive
                    dtotc / e2c / wcols for all chunks in one go."""
                    pb = ps_tiny.tile([P, 2 * NCH * H], dt.float32, tag="tiny")
                    nc.tensor.matmul(
                        pb[:, 0 : NCH * H], ones_row[0:1, 0:P], ulast[:],
                        start=True, stop=True,
                    )
                    nc.tensor.matmul(
                        pb[:, NCH * H : 2 * NCH * H], ones_row[0:1, 0:P], plrow[:],
                        start=True, stop=True,
                    )
                    nc.vector.tensor_copy(dtotc[:], pb[:, 0 : NCH * H])
                    for c in range(NCH):
                        co = c * 2 * H
                        nc.vector.tensor_sub(
                            e2c[:, c * H : (c + 1) * H],
                            pb[:, NCH * H + c * H : NCH * H + (c + 1) * H],
                            cols[:, co : co + H],
                        )
                    nc.scalar.activation(e2c[:], e2c[:], AF.Exp)
                    for c in range(NCH):
                        co = c * 2 * H
                        nc.vector.tensor_mul(
                            wcols[:, c * H : (c + 1) * H],
                            e2c[:, c * H : (c + 1) * H],
                            cols[:, co + H : co + 2 * H],
                        )

                urows = {}

                def prefetch_urow(mt):
                    """Stage u rows for block mt early, so the K=1 broadcast
                    matmuls never stall behind weight transfers."""
                    if mt > 7 or mt in urows:
                        return
                    he, ho = 2 * mt, 2 * mt + 1
                    urow = wrk.tile(
                        [1, 2 * T], dt.bfloat16, tag="urow", bufs=2, name=f"urow{mt}"
                    )
                    nc.sync.dma_start(out=urow[:, 0:T], in_=u_all[he : he + 1, :])
                    nc.sync.dma_start(out=urow[:, T : 2 * T], in_=u_all[ho : ho + 1, :])
                    urows[mt] = urow

                def emit_cw(mt):
                    """cw = cm * u (broadcast u rows via K=1 matmuls, cast,
                    then scale the repacked cm in SBUF)."""
                    he, ho = 2 * mt, 2 * mt + 1
                    prefetch_urow(mt)
                    urow = urows.pop(mt)
                    prefetch_urow(mt + 1)
                    ubc = wrk.tile([N, 2 * T], dt.bfloat16, tag="ubc", bufs=1)
                    for k in range(2):
                        ubp = ps_proj.tile(
                            [N, T], dt.float32, tag="proj", name=f"ubp{mt}_{k}"
                        )
                        nc.tensor.matmul(
                            ubp[:], ones_bf[0:1, 0:N], urow[:, k * T : (k + 1) * T],
                            start=True, stop=True,
                        )
                        if k == 0:
                            nc.scalar.activation(ubc[:, 0:T], ubp[:], AF.Copy)
                        else:
                            nc.vector.tensor_copy(ubc[:, T : 2 * T], ubp[:])
                    nc.gpsimd.tensor_mul(
                        cw[:, he * T : (he + 1) * T],
                        cm[:, he * T : (he + 1) * T],
                        ubc[:, 0:T],
                    )
                    nc.vector.tensor_mul(
                        cw[:, ho * T : (ho + 1) * T],
                        cm[:, ho * T : (ho + 1) * T],
                        ubc[:, T : 2 * T],
                    )

                yv = y[:].rearrange("p (h t) -> p h t", h=KT)  # (128, 16, 512)
                scan_gw = {}
                zps = [None, None]

                def emit_scan_front(hg, c):
                    """Scan group front: one K=16 E matmul + G matmuls,
                    then the exp/mask/mul chain."""
                    dbank = ps_diff.tile([P, 4 * L], dt.float32, tag="diff")
                    gbank = ps_g.tile([P, 4 * L], dt.float32, tag="g")
                    nc.tensor.matmul(
                        dbank[:],
                        lhq[c][:, hg * L : (hg + 1) * L],
                        rpq[c][:, hg * 4 * L : (hg + 1) * 4 * L],
                        start=True,
                        stop=True,
                    )
                    for hi in range(4):
                        h = hg * 4 + hi
                        hb = slice(h * T + c * L, h * T + (c + 1) * L)
                        nc.tensor.matmul(
                            gbank[:, hi * L : (hi + 1) * L],
                            bm[:, hb],
                            cm[:, hb],
                            start=True,
                            stop=True,
                        )
                    e_sb = wrk.tile([P, 4 * L], dt.bfloat16, tag="e_sb", bufs=3)
                    nc.scalar.activation(e_sb[:], dbank[:], AF.Exp)
                    # causal mask: keep i>=j else 0 (kills the exp-overflow infs)
                    nc.gpsimd.affine_select(
                        out=e_sb[:],
                        in_=e_sb[:],
                        compare_op=OP.is_ge,
                        fill=0.0,
                        base=0,
                        pattern=[[0, 4], [1, L]],
                        channel_multiplier=-1,
                    )
                    gw = wrk.tile([P, 4 * L], dt.bfloat16, tag="gw", bufs=4)
                    nc.vector.tensor_mul(gw[:], gbank[:], e_sb[:])
                    scan_gw[(hg, c)] = gw

                def emit_scan_sd(hg):
                    """State machinery for all 4 chunks of a head group.
                    Depends only on bm/x/dt-chain (NOT on the fronts), so the
                    whole state chain resolves early and the Y matmuls later
                    never wait on a serial VE chain."""
                    for c in range(NCH):
                        # btr shares the diff pool's banks (tag-shared
                        # rotation); the slot's prior dbank is drained by then.
                        btr = ps_diff.tile(
                            [P, 4 * N], dt.bfloat16, tag="diff", name=f"btr{hg}_{c}"
                        )
                        sdb = ps_sd.tile([N, 4 * P], dt.float32, tag="sd")
                        bd = wrk.tile([P, 4 * N], dt.bfloat16, tag="bd")
                        # paired transposes (both heads of a bc block in one
                        # 128-wide op, straight from the un-repacked btmp),
                        # then the sd matmuls: the bd-scale round-trip hides
                        # behind the remaining transposes.
                        cb = slice(c * L, (c + 1) * L)
                        for hi2 in range(2):
                            nc.tensor.transpose(
                                btr[:, hi2 * 2 * N : (hi2 + 1) * 2 * N],
                                btmp_b[2 * hg + hi2][:, cb],
                                ident_p[:],
                            )
                        for hi in range(4):
                            h = hg * 4 + hi
                            # bd = btr * (e2c*dt): split evacs ACT/DVE
                            wc = wcols[:, c * H + h : c * H + h + 1]
                            if hi % 2 == 0:
                                nc.scalar.activation(
                                    bd[:, hi * N : (hi + 1) * N],
                                    btr[:, hi * N : (hi + 1) * N],
                                    AF.Copy,
                                    scale=wc,
                                )
                            else:
                                nc.vector.tensor_scalar_mul(
                                    bd[:, hi * N : (hi + 1) * N],
                                    btr[:, hi * N : (hi + 1) * N],
                                    wc,
                                )
                        for hi in range(4):
                            h = hg * 4 + hi
                            xc = x[:, c * D + h * P : c * D + (h + 1) * P]
                            nc.tensor.matmul(
                                sdb[:, hi * P : (hi + 1) * P],
                                bd[:, hi * N : (hi + 1) * N], xc,
                                start=True, stop=True,
                            )
                        gs = slice(hg * 4 * P, (hg + 1) * 4 * P)
                        if c == 0:
                            # S_after_0 = sds_0 (prior state is zero)
                            nc.vector.tensor_copy(s_ck[0][0:N, gs], sdb[:])
                        elif c < NCH - 1:
                            for hi in range(4):
                                h = hg * 4 + hi
                                nc.vector.scalar_tensor_tensor(
                                    s_ck[c][0:N, h * P : (h + 1) * P],
                                    s_ck[c - 1][0:N, h * P : (h + 1) * P],
                                    dtotc[0:N, c * H + h : c * H + h + 1],
                                    sdb[:, hi * P : (hi + 1) * P],
                                    op0=OP.mult,
                                    op1=OP.add,
                                )
                        # c == NCH-1: the post-chunk-3 state is never read

                def emit_scan_y(hg, c, ps_y):
                    """Y^T = x_chunk^T Gw (+ S_prev^T cw for c>0), evac to y."""
                    gw = scan_gw.pop((hg, c))
                    cb = slice(c * L, (c + 1) * L)
                    ybank = ps_y.tile([P, 4 * L], dt.float32, tag="y")
                    for hi in range(4):
                        h = hg * 4 + hi
                        hb = slice(h * T + c * L, h * T + (c + 1) * L)
                        xc = x[:, c * D + h * P : c * D + (h + 1) * P]
                        nc.tensor.matmul(
                            ybank[:, hi * L : (hi + 1) * L],
                            xc,
                            gw[:, hi * L : (hi + 1) * L],
                            start=True,
                            stop=(c == 0),
                        )
                        if c > 0:
                            nc.tensor.matmul(
                                ybank[:, hi * L : (hi + 1) * L],
                                s_ck[c - 1][0:N, h * P : (h + 1) * P],
                                cw[:, hb],
                                start=False,
                                stop=True,
                            )
                    # Y evac: psum (p, 4*L) -> y (d,t) blocks [h, c*L:(c+1)*L]
                    nc.scalar.activation(
                        yv[:, hg * 4 : hg * 4 + 4, cb],
                        ybank[:].rearrange("p (h t) -> p h t", h=4),
                        AF.Copy,
                    )

                def emit_z_alloc():
                    zps[0] = ps_proj.tile([P, T], dt.float32, tag="proj", name="zps0")
                    zps[1] = ps_proj.tile([P, T], dt.float32, tag="proj", name="zps1")

                def emit_z(h):
                    """Accumulate head h's slice of z = W_yo^T y."""
                    for ut in range(2):
                        nc.tensor.matmul(
                            zps[ut][:],
                            w_yo[:, h * U + ut * P : h * U + (ut + 1) * P],
                            y[:, h * T : (h + 1) * T],
                            start=(h == 0),
                            stop=(h == H - 1),
                        )

                # ---------------- phase 2 schedule ----------------------
                # PE backbone: bc blocks + z; the state machinery (SD),
                # fronts (F) and Y matmuls slot between them as soon as
                # their deps land. All serial chains resolve early.
                staged[(0, "b")] = emit_bc_stage(0, "b")
                staged[(0, "c")] = emit_bc_stage(0, "c")
                staged[(1, "b")] = emit_bc_stage(1, "b")
                staged[(1, "c")] = emit_bc_stage(1, "c")
                emit_bc_proj(0)
                staged[(2, "b")] = emit_bc_stage(2, "b")
                staged[(2, "c")] = emit_bc_stage(2, "c")
                emit_packs(0)
                emit_dt_pe1()
                emit_packs(1)
                emit_bc_proj(1)
                emit_dt_pe2()
                emit_packs(2)
                emit_packs(3)
                prefetch_urow(0)

            emit_cw(0)
            emit_cw(1)
            emit_scan_sd(0)
            emit_scan_front(0, 0)
            emit_scan_front(0, 1)
            staged[(3, "b")] = emit_bc_stage(3, "b")
            staged[(3, "c")] = emit_bc_stage(3, "c")
            emit_bc_proj(2)
            emit_scan_front(0, 2)
            emit_scan_front(0, 3)
            with tc.tile_pool(name="ps_y", bufs=1, space="PSUM") as ps_y:
                emit_scan_y(0, 0, ps_y)
                emit_scan_y(0, 1, ps_y)
                staged[(4, "b")] = emit_bc_stage(4, "b")
                staged[(4, "c")] = emit_bc_stage(4, "c")
                emit_bc_proj(3)
                emit_scan_y(0, 2, ps_y)
                emit_scan_y(0, 3, ps_y)
                emit_scan_sd(1)
                emit_cw(2)
                emit_cw(3)
                emit_scan_front(1, 0)
                emit_scan_front(1, 1)
                staged[(5, "b")] = emit_bc_stage(5, "b")
                staged[(5, "c")] = emit_bc_stage(5, "c")
                emit_bc_proj(4)
                emit_scan_front(1, 2)
                emit_scan_front(1, 3)
                emit_scan_y(1, 0, ps_y)
                emit_scan_y(1, 1, ps_y)
                staged[(6, "b")] = emit_bc_stage(6, "b")
                staged[(6, "c")] = emit_bc_stage(6, "c")
                emit_bc_proj(5)
                emit_scan_y(1, 2, ps_y)
                emit_scan_y(1, 3, ps_y)
                emit_scan_sd(2)
                emit_cw(4)
                emit_cw(5)
                emit_scan_front(2, 0)
                emit_scan_front(2, 1)
                staged[(7, "b")] = emit_bc_stage(7, "b")
                staged[(7, "c")] = emit_bc_stage(7, "c")
                emit_bc_proj(6)
                emit_scan_front(2, 2)
                emit_scan_front(2, 3)
                emit_scan_y(2, 0, ps_y)
                emit_scan_y(2, 1, ps_y)
                emit_cw(6)
                emit_bc_proj(7)
                emit_scan_y(2, 2, ps_y)
                emit_scan_y(2, 3, ps_y)
                emit_scan_sd(3)
                emit_cw(7)
                emit_scan_front(3, 0)
                emit_scan_front(3, 1)
                emit_scan_front(3, 2)
                emit_scan_front(3, 3)
                emit_z_alloc()
                emit_z(0)
                emit_z(1)
                emit_z(2)
                emit_z(3)
                emit_scan_y(3, 0, ps_y)
                emit_z(4)
                emit_z(5)
                emit_scan_y(3, 1, ps_y)
                emit_z(6)
                emit_z(7)
                emit_scan_y(3, 2, ps_y)
                emit_z(8)
                emit_z(9)
                emit_z(10)
                emit_z(11)
                emit_scan_y(3, 3, ps_y)
                emit_z(12)
                emit_z(13)
                emit_z(14)
                emit_z(15)

                # ---------------- tail: zT + logits ---------------------
                zT = cst.tile([P, 2 * T], dt.bfloat16, tag="zT")  # (u, t)
                for ut in range(2):
                    if with_b_yo:
                        nc.scalar.activation(
                            zT[:, ut * T : (ut + 1) * T], zps[ut][:], AF.Relu,
                            bias=b_yo[:, ut : ut + 1],
                        )
                    else:
                        nc.scalar.activation(
                            zT[:, ut * T : (ut + 1) * T], zps[ut][:], AF.Relu
                        )

                logit = cst.tile([P, NCH * A], dt.float32, tag="logit")
                for tt in range(NCH):
                    ps = ps_y.tile([P, A], dt.float32, tag="y", name=f"lg{tt}")
                    nmm = 3 if with_b_head else 2
                    for ut in range(2):
                        nc.tensor.matmul(
                            ps[:],
                            zT[:, ut * T + tt * P : ut * T + (tt + 1) * P],
                            w_hd[:, ut * A : (ut + 1) * A],
                            start=(ut == 0),
                            stop=(ut == nmm - 1),
                        )
                    if with_b_head:
                        nc.tensor.matmul(
                            ps[:],
                            ones_bf[0:1, tt * P : (tt + 1) * P],
                            b_hd[:],
                            start=False,
                            stop=True,
                        )
                    nc.scalar.activation(logit[:, tt * A : (tt + 1) * A], ps[:], AF.Copy)
                    nc.sync.dma_start(
                        out=out_e[tt * P : (tt + 1) * P, :],
                        in_=logit[:, tt * A : (tt + 1) * A],
                    )

    _split_multi_waits(nc)
    return nc


def kernel(obs, W_in, b_in, A_log, dt_bias, W_dt, W_B, W_C, W_yo, b_yo, W_head, b_head):
    _inject_axon_hooks()
    _patch_tile()
    from concourse.bass_utils import run_bass_kernel_spmd

    obs = np.asarray(obs, dtype=np.float32)
    flags = (
        bool(np.any(np.asarray(b_in) != 0)),
        bool(np.any(np.asarray(b_yo) != 0)),
        bool(np.any(np.asarray(b_head) != 0)),
    )
    # First call: build once (the verified path). Repeat calls in one
    # process rebuild a fresh graph — re-executing a previously-run nc with
    # new inputs has crashed the exec unit (NRT status 101) in testing.
    if flags not in _CACHE:
        _CACHE[flags] = _build(*flags)
    elif _EXECUTED.get(flags):
        _CACHE[flags] = _build(*flags)
    nc = _CACHE[flags]
    _EXECUTED[flags] = True

    obsT = obs.reshape(T, BSZ, OBSD).transpose(1, 2, 0)  # (B, 256, T)

    def colblocks(w, blk):
        # (D, M) -> (P, M//blk * KT * blk): per-partition contiguous blocks
        m = w.shape[1]
        return np.ascontiguousarray(
            w.reshape(KT, P, m // blk, blk).transpose(1, 2, 0, 3).reshape(P, -1)
        )

    base = {
        "w_in": np.ascontiguousarray(W_in).astype(BF16),
        "w_dt": colblocks(np.asarray(W_dt), H).astype(BF16),
        "w_b": colblocks(np.asarray(W_B), P).astype(BF16),
        "w_c": colblocks(np.asarray(W_C), P).astype(BF16),
        "w_yo": colblocks(np.asarray(W_yo), U).astype(BF16),
        "w_hd": np.ascontiguousarray(
            np.asarray(W_head).reshape(2, P, A).transpose(1, 0, 2).reshape(P, 2 * A)
        ).astype(BF16),
        "neg_a": (-np.exp(np.asarray(A_log, np.float64)))
        .astype(np.float32)
        .reshape(H, 1),
        "dtb": np.asarray(dt_bias, np.float32).reshape(H, 1),
    }
    if flags[0]:
        base["b_in"] = np.ascontiguousarray(
            np.asarray(b_in, np.float32).reshape(KT, P).T
        )
    if flags[1]:
        base["b_yo"] = np.ascontiguousarray(
            np.asarray(b_yo, np.float32).reshape(2, P).T
        )
    if flags[2]:
        base["b_hd"] = np.asarray(b_head).astype(BF16).reshape(1, A)
    in_maps = [
        dict(base, obsT=np.ascontiguousarray(obsT[c]).astype(BF16)) for c in range(BSZ)
    ]
    global _last_in_maps
    _last_in_maps = in_maps
    res = run_bass_kernel_spmd(nc, in_maps, core_ids=list(range(BSZ)))
    out = np.stack([res.results[c]["out"] for c in range(BSZ)], axis=1)
    return out.astype(np.float32)


# revision 31
# speedup vs baseline: 1.1985x; 1.1287x over previous
"""Self-contained Trainium2 kernel for the SSD-scan actor network.

Data-parallel over batch B=8 across 8 NeuronCores (one sample per core, no
collectives). Per core:
  x  = relu(obs @ W_in + b_in)                  (T=512, D=2048)
  dt = softplus(x @ W_dt + dt_bias)             (T, H=16)
  Bm = x @ W_B, Cm = x @ W_C                    (T, H, N=64)
  y  = selective scan over T (Mamba2 SSD)       (T, D)
  z  = relu(y @ W_yo + b_yo)                    (T, U=256)
  out = z @ W_head + b_head                     (T, A=64)

The scan uses the chunked (segsum) SSD formulation: chunk length L=128,
4 chunks, 16 independent heads. Per head/chunk:
  E[j,i]  = exp(Pcum_i - Pcum_j + log dt_j), causally masked to j<=i
  Y^T     = x_chunk^T Gw + S_prev^T (C*u),  Gw = (B C^T)^T . E, u_i=exp(Pcum_i)
  S_new   = exp(Pcum_L-1) S_prev + sum_j exp(Pcum_L-1 - Pcum_j) dt_j B_j x_j^T
Big matmuls run in bf16 with fp32 PSUM accumulation; the Diff matrix
(Pcum_i - Pcum_j + logdt_j) is built exactly with K=4 bf16 (hi/lo) matmuls.

v8 schedule: the PE instruction stream is kept dense end-to-end so the HAM
activity monitor never demotes the clock to 1.2 GHz:
  - weights are pre-transposed on the host so every SBUF load is one
    contiguous descriptor per partition (the old strided loads saturated
    the DMA/sync engines);
  - the dt chain is batched across all 4 chunks (cumsum via the DVE
    tensor_tensor_scan op) so its PE footprint is ~10 small ops that are
    slotted between the B/C projection blocks instead of serializing them;
  - scan groups start as soon as their B/C block lands; the z projection
    and the last head-group's chunk chain share the tail.

Hardware notes (all discovered the hard way on this container's stack):
  - walrus here allows only ONE sync wait per instruction -> _split_multi_waits
  - matmul operands whose APs start at partition 64 crash the exec unit
    (NRT_EXEC_UNIT_UNRECOVERABLE), so every matmul operand is kept at base
    partition 0: B/C are repacked to 64-partition tensors via bf16
    staging + SBUF-to-SBUF DMA (DMA moves across partitions; DVE cannot).
  - Softplus shares no ACT function table with Exp/Ln -> ln(1+exp(x)).
"""

import sys
import types

import numpy as np
import ml_dtypes

T, BSZ, OBSD = 512, 8, 256
D, H, N, P = 2048, 16, 64, 128
U, A = 256, 64
L, NCH, KT = 128, 4, 16  # chunk length, #chunks, #d-tiles (D/128)
MT = 8  # B/C column blocks (HN/128)
BF16 = ml_dtypes.bfloat16

_CACHE = {}
_EXECUTED = {}


def _patch_tile():
    """Split the TileContext final drain's waits across single-wait nops."""
    from concourse import tile, mybir
    from concourse.vector_clock import ScopedClock

    if getattr(tile.TileContext, "_drain_patched", False):
        return

    def _patched(self, tick_clock, wait_clock):
        nc = self.nc
        probe = nc.sync.nop()
        wait_clock.add_sem_waits(
            probe.ins, ScopedClock({None: tick_clock.global_clock})
        )
        si = probe.ins.sync_info
        if si is not None and len(si.on_wait) > 1:
            waits = list(si.on_wait)
            probe.ins.sync_info = mybir.SyncInfo(
                on_wait=[waits[0]], on_update=list(si.on_update)
            )
            for w in waits[1:]:
                nop = nc.sync.nop()
                nop.ins.sync_info = mybir.SyncInfo(on_wait=[w], on_update=[])
        nc.sync.drain()
        nc.all_engine_barrier(sem_only=True)
        assert self.sems is not None
        popped = nc._tile_sem_poison_stack.pop()
        assert popped is self._sem_poison
        nc.clear_and_free_semaphores(list(self.sems.allocated().values()))
        # NOTE: the stock drain runs a second all_engine_barrier here (~5us
        # of ring latency); nothing uses the cleared semaphores afterwards —
        # the NEFF ends — so it is skipped.

    tile.TileContext._drain_and_barrier = _patched
    tile.TileContext._drain_patched = True


def _split_multi_waits(nc):
    """This walrus build accepts at most one sync wait per instruction.
    Hoist extra waits onto single-wait NoOps inserted just before, on the
    same engine (the sequencer stalls there first — strictly conservative)."""
    from concourse import mybir

    n = 0
    for f in nc.m.functions:
        for bb in f.blocks:
            insts = list(bb.instructions)
            changed = False
            new = []
            for inst in insts:
                try:
                    si = inst.sync_info
                except Exception:
                    si = None
                if si is not None and len(si.on_wait) > 1:
                    waits = list(si.on_wait)
                    for w in waits[:-1]:
                        nop = mybir.InstNoOp(
                            name=f"wsplit-{n}", ins=[], outs=[], engine=inst.engine
                        )
                        n += 1
                        nop.sync_info = mybir.SyncInfo(on_wait=[w], on_update=[])
                        nc.register_instruction(nop, overwrite=True)
                        new.append(nop)
                    inst.sync_info = mybir.SyncInfo(
                        on_wait=[waits[-1]], on_update=list(si.on_update)
                    )
                    changed = True
                new.append(inst)
            if changed:
                bb.instructions = new


def _inject_axon_hooks():
    """Make trace=True work (and a BASS_TRACE env var safe) in this container."""
    if "antenv.axon_hooks" not in sys.modules:
        try:
            from trn_agent_boot.trn_boot import _ntff_profile_via_ctypes

            hook = _ntff_profile_via_ctypes("/opt/axon/libaxon_pjrt.so")
        except Exception:
            hook = None
        mod = types.ModuleType("antenv.axon_hooks")
        mod.get_axon_ntff_profile_hook = lambda: hook
        mod.set_axon_ntff_profile_hook = lambda h: None
        sys.modules["antenv.axon_hooks"] = mod
    from concourse import bass_utils

    bass_utils.upload_artifacts = lambda d: d


def _build(with_b_in, with_b_yo, with_b_head):
    import concourse.bass as bass
    import concourse.mybir as mybir
    from concourse.tile import TileContext
    from concourse.masks import make_identity

    dt = mybir.dt
    AF = mybir.ActivationFunctionType
    OP = mybir.AluOpType

    nc = bass.Bass()
    obsT_e = nc.declare_dram_parameter("obsT", [OBSD, T], dt.bfloat16, isOutput=False)
    w_in_e = nc.declare_dram_parameter("w_in", [OBSD, D], dt.bfloat16, isOutput=False)
    # pre-transposed on the host: one contiguous run per partition per block
    w_dt_e = nc.declare_dram_parameter("w_dt", [P, KT * H], dt.bfloat16, isOutput=False)
    w_b_e = nc.declare_dram_parameter("w_b", [P, MT * KT * P], dt.bfloat16, isOutput=False)
    w_c_e = nc.declare_dram_parameter("w_c", [P, MT * KT * P], dt.bfloat16, isOutput=False)
    w_yo_e = nc.declare_dram_parameter("w_yo", [P, KT * U], dt.bfloat16, isOutput=False)
    w_hd_e = nc.declare_dram_parameter("w_hd", [P, 2 * A], dt.bfloat16, isOutput=False)
    neg_a_e = nc.declare_dram_parameter("neg_a", [H, 1], dt.float32, isOutput=False)
    dtb_e = nc.declare_dram_parameter("dtb", [H, 1], dt.float32, isOutput=False)
    bin_e = byo_e = bhd_e = None
    if with_b_in:
        bin_e = nc.declare_dram_parameter("b_in", [P, KT], dt.float32, isOutput=False)
    if with_b_yo:
        byo_e = nc.declare_dram_parameter("b_yo", [P, 2], dt.float32, isOutput=False)
    if with_b_head:
        bhd_e = nc.declare_dram_parameter("b_hd", [1, A], dt.bfloat16, isOutput=False)
    out_e = nc.declare_dram_parameter("out", [T, A], dt.float32, isOutput=True)

    _patch_tile()
    with TileContext(nc) as tc:
        with (
            tc.tile_pool(name="cst", bufs=1) as cst,
            tc.tile_pool(name="wrk", bufs=2) as wrk,
            tc.tile_pool(name="ps_proj", bufs=2, space="PSUM") as ps_proj,
            tc.tile_pool(name="ps_diff", bufs=2, space="PSUM") as ps_diff,
            tc.tile_pool(name="ps_g", bufs=2, space="PSUM") as ps_g,
            tc.tile_pool(name="ps_sd", bufs=1, space="PSUM") as ps_sd,
        ):
            # ---------------- phase 0: warm-up + first DMAs ----------
            obsT = cst.tile([P, 2 * T], dt.bfloat16, tag="obsT")
            for k in range(2):
                nc.sync.dma_start(
                    out=obsT[:, k * T : (k + 1) * T], in_=obsT_e[k * P : (k + 1) * P, :]
                )
            w_in = cst.tile([P, 2 * D], dt.bfloat16, tag="w_in")
            for k in range(2):
                nc.sync.dma_start(
                    out=w_in[:, k * D : (k + 1) * D], in_=w_in_e[k * P : (k + 1) * P, :]
                )
            w_dt = cst.tile([P, KT * H], dt.bfloat16, tag="w_dt")
            nc.sync.dma_start(out=w_dt[:], in_=w_dt_e[:])
            neg_a = cst.tile([H, 1], dt.float32, tag="neg_a")
            nc.sync.dma_start(out=neg_a[:], in_=neg_a_e[:])
            dtb = cst.tile([H, 1], dt.float32, tag="dtb")
            nc.sync.dma_start(out=dtb[:], in_=dtb_e[:])
            if with_b_in:
                b_in = cst.tile([P, KT], dt.float32, tag="b_in")
                nc.sync.dma_start(out=b_in[:], in_=bin_e[:])

            # HAM warm-up: PE activity from ~t0 flips the clock gate to
            # 2.4 GHz while obs/W_in stream in. The y tile is garbage at
            # this point — the accumulating PSUM bank is never read, and
            # y's real writers (ACT evacs, ~45us in) come long after.
            y = cst.tile([P, KT * T], dt.bfloat16, tag="y")  # (d, t)
            with tc.tile_pool(name="ps_tiny", bufs=1, space="PSUM") as ps_tiny:
                # single accumulating bank: no PSUM rotation waits, so the
                # warm matmuls stream back-to-back from t~0
                pw = ps_proj.tile([P, T], dt.float32, tag="proj", name="warm")
                for i in range(18):
                    nc.tensor.matmul(
                        pw[:], y[:, 0:P], y[:, 0:T], start=(i == 0), stop=(i == 17)
                    )

                # constants (gpsimd/PE while warm-up runs)
                ident_f = cst.tile([P, P], dt.float32, tag="ident_f")
                make_identity(nc, ident_f[:])
                ident_b = cst.tile([N, N], dt.bfloat16, tag="ident_b")
                make_identity(nc, ident_b[:])
                ident_p = cst.tile([P, P], dt.bfloat16, tag="ident_p")
                make_identity(nc, ident_p[:])
                ones_row = cst.tile([1, P], dt.float32, tag="ones_row")
                nc.gpsimd.memset(ones_row[:], 1.0)
                ones_bf = cst.tile([1, T], dt.bfloat16, tag="ones_bf")
                nc.gpsimd.memset(ones_bf[:], 1.0)
                zero_hl = cst.tile([H, L], dt.float32, tag="zero_hl")
                nc.gpsimd.memset(zero_hl[:], 0.0)

                # ---------------- phase 1: in-proj + dt-proj ------------
                # x^T = relu(W_in^T obs^T) (d,t), 16 d-tiles; the dt
                # projection matmuls ride two slots behind their relu.
                xT = cst.tile([P, KT * T], dt.bfloat16, tag="xT")  # (d, t)
                psd = ps_tiny.tile([H, T], dt.float32, tag="tiny", name="psd")

                def emit_inproj(kt):
                    ps = ps_proj.tile([P, T], dt.float32, tag="proj")
                    for ko in range(2):
                        nc.tensor.matmul(
                            ps[:],
                            w_in[:, ko * D + kt * P : ko * D + (kt + 1) * P],
                            obsT[:, ko * T : (ko + 1) * T],
                            start=(ko == 0),
                            stop=(ko == 1),
                        )
                    if with_b_in:
                        nc.scalar.activation(
                            xT[:, kt * T : (kt + 1) * T], ps[:], AF.Relu,
                            bias=b_in[:, kt : kt + 1],
                        )
                    else:
                        nc.scalar.activation(xT[:, kt * T : (kt + 1) * T], ps[:], AF.Relu)

                def emit_dtproj(kt):
                    nc.tensor.matmul(
                        psd[:],
                        w_dt[:, kt * H : (kt + 1) * H],
                        xT[:, kt * T : (kt + 1) * T],
                        start=(kt == 0),
                        stop=(kt == KT - 1),
                    )

                emit_inproj(0)
                emit_inproj(1)
                for kt in range(2, KT):
                    emit_inproj(kt)
                    emit_dtproj(kt - 2)
                emit_dtproj(KT - 2)
                emit_dtproj(KT - 1)

                # ---------------- dt chain, part A (VE/ACT/DMA only) ----
                # softplus via ln(1+exp(.)) — Softplus shares no ACT table
                # with Exp/Ln here; exp/ln/relu/copy live in one table.
                dtraw = cst.tile([H, T], dt.float32, tag="dtraw")
                dtT = cst.tile([H, T], dt.float32, tag="dtT")
                pcumT = cst.tile([H, T], dt.float32, tag="pcumT")
                u_all = cst.tile([H, T], dt.bfloat16, tag="u_all")  # exp(Pcum)
                pcumH = cst.tile([H, T], dt.bfloat16, tag="pcumH")
                pcumL = cst.tile([H, T], dt.bfloat16, tag="pcumL")
                npdH = cst.tile([H, T], dt.bfloat16, tag="npdH")
                npdL = cst.tile([H, T], dt.bfloat16, tag="npdL")

                nc.scalar.activation(dtraw[:], psd[:], AF.Exp, bias=dtb[:])
                nc.vector.tensor_scalar_add(dtraw[:], dtraw[:], 1.0)
                nc.scalar.activation(dtT[:], dtraw[:], AF.Ln)
                # ldec lives in pcumT's buffer; the scan runs in place
                nc.vector.tensor_scalar_mul(pcumT[:], dtT[:], neg_a[:])
                for c in range(NCH):
                    cb = slice(c * L, (c + 1) * L)
                    nc.vector.tensor_tensor_scan(
                        pcumT[:, cb], pcumT[:, cb], zero_hl[:], 0.0,
                        op0=OP.add, op1=OP.add,
                    )
                nc.scalar.activation(u_all[:], pcumT[:], AF.Exp)
                logdt = dtraw  # dtraw is dead after dtT; reuse its slot
                nc.scalar.activation(logdt[:], dtT[:], AF.Ln)
                nc.vector.tensor_sub(logdt[:], logdt[:], pcumT[:])  # now -Pcum+logdt
                nc.vector.tensor_copy(pcumH[:], pcumT[:])
                nc.vector.tensor_sub(pcumL[:], pcumT[:], pcumH[:])
                nc.vector.tensor_copy(npdH[:], logdt[:])
                nc.vector.tensor_sub(npdL[:], logdt[:], npdH[:])

                # persistent per-chunk diff-pack tiles: all 16 heads side by
                # side, so scan fronts need no DMAs at all. lh rows
                # [1, npdH, 1, npdL]; rp [pcumH, 1, pcumL, 1] (the "ones"
                # rows survive from the full-tile memset).
                lhc = [
                    cst.tile([4, H * L], dt.bfloat16, tag=f"lhc{c}", name=f"lhc{c}")
                    for c in range(NCH)
                ]
                rpc = [
                    cst.tile([4, H * L], dt.bfloat16, tag=f"rpc{c}", name=f"rpc{c}")
                    for c in range(NCH)
                ]
                for c in range(NCH):
                    nc.vector.memset(lhc[c][:], 1.0)
                    nc.vector.memset(rpc[c][:], 1.0)

                def emit_packs(c):
                    cb = slice(c * L, (c + 1) * L)
                    nc.sync.dma_start(
                        out=rpc[c][0:1, :].rearrange("p (h t) -> p h t", h=H),
                        in_=pcumH[:, cb],
                    )
                    nc.sync.dma_start(
                        out=rpc[c][2:3, :].rearrange("p (h t) -> p h t", h=H),
                        in_=pcumL[:, cb],
                    )
                    nc.sync.dma_start(
                        out=lhc[c][1:2, :].rearrange("p (h t) -> p h t", h=H),
                        in_=npdH[:, cb],
                    )
                    nc.sync.dma_start(
                        out=lhc[c][3:4, :].rearrange("p (h t) -> p h t", h=H),
                        in_=npdL[:, cb],
                    )

                # ---------------- x (t,d) via PE transposes -------------
                x = cst.tile([P, NCH * D], dt.bfloat16, tag="x")  # (t, d)
                for tt in range(NCH):
                    for dg in range(4):
                        psx = ps_proj.tile([P, 4 * P], dt.bfloat16, tag="proj")
                        for k4 in range(4):
                            kt = dg * 4 + k4
                            nc.tensor.transpose(
                                psx[:, k4 * P : (k4 + 1) * P],
                                xT[:, kt * T + tt * P : kt * T + (tt + 1) * P],
                                ident_p[:],
                            )
                        nc.vector.tensor_copy(
                            x[:, tt * D + dg * 512 : tt * D + (dg + 1) * 512], psx[:]
                        )

                # late weight DMAs (off the critical path)
                w_yo = cst.tile([P, KT * U], dt.bfloat16, tag="w_yo")
                nc.sync.dma_start(out=w_yo[:], in_=w_yo_e[:])
                w_hd = cst.tile([P, 2 * A], dt.bfloat16, tag="w_hd")
                nc.sync.dma_start(out=w_hd[:], in_=w_hd_e[:])
                if with_b_yo:
                    b_yo = cst.tile([P, 2], dt.float32, tag="b_yo")
                    nc.sync.dma_start(out=b_yo[:], in_=byo_e[:])
                if with_b_head:
                    b_hd = cst.tile([1, A], dt.bfloat16, tag="b_hd")
                    nc.sync.dma_start(out=b_hd[:], in_=bhd_e[:])

                # ---------------- scan state/staging tensors ------------
                bm = cst.tile([N, H * T], dt.bfloat16, tag="bm")
                cm = cst.tile([N, H * T], dt.bfloat16, tag="cm")
                cw = cst.tile([N, H * T], dt.bfloat16, tag="cw")
                # state snapshots after chunks 0/1/2 (chunk -1 state is zero,
                # the post-chunk-3 state is never read): packed (n, h*p)
                s_ck = [
                    cst.tile([N, H * P], dt.bfloat16, tag=f"sck{c}", name=f"sck{c}")
                    for c in range(NCH - 1)
                ]

                cols = cst.tile([P, NCH * 2 * H], dt.float32, tag="cols")
                plrow = cst.tile([1, NCH * H], dt.float32, tag="plrow")
                ulast = cst.tile([1, NCH * H], dt.float32, tag="ulast")
                dtotc = cst.tile([P, NCH * H], dt.float32, tag="dtotc")
                e2c = cst.tile([P, NCH * H], dt.float32, tag="e2c")
                wcols = cst.tile([P, NCH * H], dt.float32, tag="wcols")

                # ---------------- emitters ------------------------------
                def emit_bc_stage(mt, which):
                    """Issue the weight-block DMA for bc block mt (1 descriptor
                    per partition thanks to the host-side pre-transpose)."""
                    src = w_b_e if which == "b" else w_c_e
                    buf = wrk.tile(
                        [P, KT * P], dt.bfloat16, tag="wstage", bufs=2,
                        name=f"wst_{which}{mt}",
                    )
                    nc.sync.dma_start(
                        out=buf[:], in_=src[:, mt * KT * P : (mt + 1) * KT * P]
                    )
                    return buf

                staged = {}

                btmp_b = {}

                def emit_bc_proj(mt):
                    """Project W_B / W_C columns for heads (2mt, 2mt+1), repack."""
                    he, ho = 2 * mt, 2 * mt + 1
                    for which, dst in (("b", bm), ("c", cm)):
                        buf = staged.pop((mt, which))
                        ps = ps_proj.tile([P, T], dt.float32, tag="proj")
                        for kt in range(KT):
                            nc.tensor.matmul(
                                ps[:],
                                buf[:, kt * P : (kt + 1) * P],
                                xT[:, kt * T : (kt + 1) * T],
                                start=(kt == 0),
                                stop=(kt == KT - 1),
                            )
                        tmp = wrk.tile(
                            [P, T], dt.bfloat16, tag="bctmp", bufs=4,
                            name=f"{which}tmp{mt}",
                        )
                        nc.scalar.activation(tmp[:], ps[:], AF.Copy)
                        if which == "b":
                            # the stacked head-pair layout feeds the paired
                            # btr transposes in emit_scan_sd directly
                            btmp_b[mt] = tmp
                        nc.sync.dma_start(
                            out=dst[:, he * T : (he + 1) * T], in_=tmp[0:N, :]
                        )
                        nc.sync.dma_start(
                            out=dst[:, ho * T : (ho + 1) * T], in_=tmp[N:P, :]
                        )

                def emit_dt_pe1():
                    """cols: transposed (Pcum | dt) columns for all chunks."""
                    pt = ps_tiny.tile([P, 2 * NCH * H], dt.float32, tag="tiny")
                    for c in range(NCH):
                        cb = slice(c * L, (c + 1) * L)
                        nc.tensor.transpose(
                            pt[:, c * 2 * H : c * 2 * H + H],
                            pcumT[:, cb], ident_f[0:H, 0:H],
                        )
                        nc.tensor.transpose(
                            pt[:, c * 2 * H + H : (c + 1) * 2 * H],
                            dtT[:, cb], ident_f[0:H, 0:H],
                        )
                    nc.vector.tensor_copy(cols[:], pt[:])
                    # PcumLast per head at base partition 0 (row 127 of PcumCol)
                    nc.sync.dma_start(
                        out=plrow[:].rearrange("p (c h) -> p c h", c=NCH),
                        in_=cols[L - 1 : L, :].rearrange(
                            "p (c kh) -> p c kh", c=NCH
                        )[:, :, 0:H],
                    )
                    nc.scalar.activation(ulast[:], plrow[:], AF.Exp)

                def emit_dt_pe2():
                    """Broadcast ulast/plast down 128 partitions; derive
                    dtotc / e2c / wcols for all chunks in one go."""
                    pb = ps_tiny.tile([P, 2 * NCH * H], dt.float32, tag="tiny")
                    nc.tensor.matmul(
                        pb[:, 0 : NCH * H], ones_row[0:1, 0:P], ulast[:],
                        start=True, stop=True,
                    )
                    nc.tensor.matmul(
                        pb[:, NCH * H : 2 * NCH * H], ones_row[0:1, 0:P], plrow[:],
                        start=True, stop=True,
                    )
                    nc.vector.tensor_copy(dtotc[:], pb[:, 0 : NCH * H])
                    for c in range(NCH):
                        co = c * 2 * H
                        nc.vector.tensor_sub(
                            e2c[:, c * H : (c + 1) * H],
                            pb[:, NCH * H + c * H : NCH * H + (c + 1) * H],
                            cols[:, co : co + H],
                        )
                    nc.scalar.activation(e2c[:], e2c[:], AF.Exp)
                    for c in range(NCH):
                        co = c * 2 * H
                        nc.vector.tensor_mul(
                            wcols[:, c * H : (c + 1) * H],
                            e2c[:, c * H : (c + 1) * H],
                            cols[:, co + H : co + 2 * H],
                        )

                urows = {}

                def prefetch_urow(mt):
                    """Stage u rows for block mt early, so the K=1 broadcast
                    matmuls never stall behind weight transfers."""
                    if mt > 7 or mt in urows:
                        return
                    he, ho = 2 * mt, 2 * mt + 1
                    urow = wrk.tile(
                        [1, 2 * T], dt.bfloat16, tag="urow", bufs=2, name=f"urow{mt}"
                    )
                    nc.sync.dma_start(out=urow[:, 0:T], in_=u_all[he : he + 1, :])
                    nc.sync.dma_start(out=urow[:, T : 2 * T], in_=u_all[ho : ho + 1, :])
                    urows[mt] = urow

                def emit_cw(mt):
                    """cw = cm * u (broadcast u rows via K=1 matmuls, cast,
                    then scale the repacked cm in SBUF)."""
                    he, ho = 2 * mt, 2 * mt + 1
                    prefetch_urow(mt)
                    urow = urows.pop(mt)
                    prefetch_urow(mt + 1)
                    ubc = wrk.tile([N, 2 * T], dt.bfloat16, tag="ubc", bufs=1)
                    for k in range(2):
                        ubp = ps_proj.tile(
                            [N, T], dt.float32, tag="proj", name=f"ubp{mt}_{k}"
                        )
                        nc.tensor.matmul(
                            ubp[:], ones_bf[0:1, 0:N], urow[:, k * T : (k + 1) * T],
                            start=True, stop=True,
                        )
                        if k == 0:
                            nc.scalar.activation(ubc[:, 0:T], ubp[:], AF.Copy)
                        else:
                            nc.vector.tensor_copy(ubc[:, T : 2 * T], ubp[:])
                    nc.gpsimd.tensor_mul(
                        cw[:, he * T : (he + 1) * T],
                        cm[:, he * T : (he + 1) * T],
                        ubc[:, 0:T],
                    )
                    nc.vector.tensor_mul(
                        cw[:, ho * T : (ho + 1) * T],
                        cm[:, ho * T : (ho + 1) * T],
                        ubc[:, T : 2 * T],
                    )

                yv = y[:].rearrange("p (h t) -> p h t", h=KT)  # (128, 16, 512)
                scan_gw = {}
                zps = [None, None]

                def emit_scan_front(hg, c):
                    """Scan group front: diff + G matmuls, exp/mask/mul chain."""
                    dbank = ps_diff.tile([P, 4 * L], dt.float32, tag="diff")
                    gbank = ps_g.tile([P, 4 * L], dt.float32, tag="g")
                    for hi in range(4):
                        h = hg * 4 + hi
                        hb = slice(h * T + c * L, h * T + (c + 1) * L)
                        nc.tensor.matmul(
                            dbank[:, hi * L : (hi + 1) * L],
                            lhc[c][:, h * L : (h + 1) * L],
                            rpc[c][:, h * L : (h + 1) * L],
                            start=True,
                            stop=True,
                        )
                        nc.tensor.matmul(
                            gbank[:, hi * L : (hi + 1) * L],
                            bm[:, hb],
                            cm[:, hb],
                            start=True,
                            stop=True,
                        )
                    e_sb = wrk.tile([P, 4 * L], dt.bfloat16, tag="e_sb", bufs=2)
                    nc.scalar.activation(e_sb[:], dbank[:], AF.Exp)
                    # causal mask: keep i>=j else 0 (kills the exp-overflow infs)
                    nc.gpsimd.affine_select(
                        out=e_sb[:],
                        in_=e_sb[:],
                        compare_op=OP.is_ge,
                        fill=0.0,
                        base=0,
                        pattern=[[0, 4], [1, L]],
                        channel_multiplier=-1,
                    )
                    gw = wrk.tile([P, 4 * L], dt.bfloat16, tag="gw", bufs=3)
                    nc.vector.tensor_mul(gw[:], gbank[:], e_sb[:])
                    scan_gw[(hg, c)] = gw

                def emit_scan_sd(hg):
                    """State machinery for all 4 chunks of a head group.
                    Depends only on bm/x/dt-chain (NOT on the fronts), so the
                    whole state chain resolves early and the Y matmuls later
                    never wait on a serial VE chain."""
                    for c in range(NCH):
                        # btr shares the diff pool's banks (tag-shared
                        # rotation); the slot's prior dbank is drained by then.
                        btr = ps_diff.tile(
                            [P, 4 * N], dt.bfloat16, tag="diff", name=f"btr{hg}_{c}"
                        )
                        sdb = ps_sd.tile([N, 4 * P], dt.float32, tag="sd")
                        bd = wrk.tile([P, 4 * N], dt.bfloat16, tag="bd")
                        # paired transposes (both heads of a bc block in one
                        # 128-wide op, straight from the un-repacked btmp),
                        # then the sd matmuls: the bd-scale round-trip hides
                        # behind the remaining transposes.
                        cb = slice(c * L, (c + 1) * L)
                        for hi2 in range(2):
                            nc.tensor.transpose(
                                btr[:, hi2 * 2 * N : (hi2 + 1) * 2 * N],
                                btmp_b[2 * hg + hi2][:, cb],
                                ident_p[:],
                            )
                        for hi in range(4):
                            h = hg * 4 + hi
                            # bd = btr * (e2c*dt): split evacs ACT/DVE
                            wc = wcols[:, c * H + h : c * H + h + 1]
                            if hi % 2 == 0:
                                nc.scalar.activation(
                                    bd[:, hi * N : (hi + 1) * N],
                                    btr[:, hi * N : (hi + 1) * N],
                                    AF.Copy,
                                    scale=wc,
                                )
                            else:
                                nc.vector.tensor_scalar_mul(
                                    bd[:, hi * N : (hi + 1) * N],
                                    btr[:, hi * N : (hi + 1) * N],
                                    wc,
                                )
                        for hi in range(4):
                            h = hg * 4 + hi
                            xc = x[:, c * D + h * P : c * D + (h + 1) * P]
                            nc.tensor.matmul(
                                sdb[:, hi * P : (hi + 1) * P],
                                bd[:, hi * N : (hi + 1) * N], xc,
                                start=True, stop=True,
                            )
                        gs = slice(hg * 4 * P, (hg + 1) * 4 * P)
                        if c == 0:
                            # S_after_0 = sds_0 (prior state is zero)
                            nc.vector.tensor_copy(s_ck[0][0:N, gs], sdb[:])
                        elif c < NCH - 1:
                            for hi in range(4):
                                h = hg * 4 + hi
                                nc.vector.scalar_tensor_tensor(
                                    s_ck[c][0:N, h * P : (h + 1) * P],
                                    s_ck[c - 1][0:N, h * P : (h + 1) * P],
                                    dtotc[0:N, c * H + h : c * H + h + 1],
                                    sdb[:, hi * P : (hi + 1) * P],
                                    op0=OP.mult,
                                    op1=OP.add,
                                )
                        # c == NCH-1: the post-chunk-3 state is never read

                def emit_scan_y(hg, c, ps_y):
                    """Y^T = x_chunk^T Gw (+ S_prev^T cw for c>0), evac to y."""
                    gw = scan_gw.pop((hg, c))
                    cb = slice(c * L, (c + 1) * L)
                    ybank = ps_y.tile([P, 4 * L], dt.float32, tag="y")
                    for hi in range(4):
                        h = hg * 4 + hi
                        hb = slice(h * T + c * L, h * T + (c + 1) * L)
                        xc = x[:, c * D + h * P : c * D + (h + 1) * P]
                        nc.tensor.matmul(
                            ybank[:, hi * L : (hi + 1) * L],
                            xc,
                            gw[:, hi * L : (hi + 1) * L],
                            start=True,
                            stop=(c == 0),
                        )
                        if c > 0:
                            nc.tensor.matmul(
                                ybank[:, hi * L : (hi + 1) * L],
                                s_ck[c - 1][0:N, h * P : (h + 1) * P],
                                cw[:, hb],
                                start=False,
                                stop=True,
                            )
                    # Y evac: psum (p, 4*L) -> y (d,t) blocks [h, c*L:(c+1)*L]
                    nc.scalar.activation(
                        yv[:, hg * 4 : hg * 4 + 4, cb],
                        ybank[:].rearrange("p (h t) -> p h t", h=4),
                        AF.Copy,
                    )

                def emit_z_alloc():
                    zps[0] = ps_proj.tile([P, T], dt.float32, tag="proj", name="zps0")
                    zps[1] = ps_proj.tile([P, T], dt.float32, tag="proj", name="zps1")

                def emit_z(h):
                    """Accumulate head h's slice of z = W_yo^T y."""
                    for ut in range(2):
                        nc.tensor.matmul(
                            zps[ut][:],
                            w_yo[:, h * U + ut * P : h * U + (ut + 1) * P],
                            y[:, h * T : (h + 1) * T],
                            start=(h == 0),
                            stop=(h == H - 1),
                        )

                # ---------------- phase 2 schedule ----------------------
                # PE backbone: bc blocks + z; the state machinery (SD),
                # fronts (F) and Y matmuls slot between them as soon as
                # their deps land. All serial chains resolve early.
                staged[(0, "b")] = emit_bc_stage(0, "b")
                staged[(0, "c")] = emit_bc_stage(0, "c")
                staged[(1, "b")] = emit_bc_stage(1, "b")
                staged[(1, "c")] = emit_bc_stage(1, "c")
                emit_bc_proj(0)
                staged[(2, "b")] = emit_bc_stage(2, "b")
                staged[(2, "c")] = emit_bc_stage(2, "c")
                emit_packs(0)
                emit_dt_pe1()
                emit_packs(1)
                emit_bc_proj(1)
                emit_dt_pe2()
                emit_packs(2)
                emit_packs(3)
                prefetch_urow(0)

            emit_cw(0)
            emit_cw(1)
            emit_scan_sd(0)
            emit_scan_front(0, 0)
            emit_scan_front(0, 1)
            staged[(3, "b")] = emit_bc_stage(3, "b")
            staged[(3, "c")] = emit_bc_stage(3, "c")
            emit_bc_proj(2)
            emit_scan_front(0, 2)
            emit_scan_front(0, 3)
            with tc.tile_pool(name="ps_y", bufs=1, space="PSUM") as ps_y:
                emit_scan_y(0, 0, ps_y)
                emit_scan_y(0, 1, ps_y)
                staged[(4, "b")] = emit_bc_stage(4, "b")
                staged[(4, "c")] = emit_bc_stage(4, "c")
                emit_bc_proj(3)
                emit_scan_y(0, 2, ps_y)
                emit_scan_y(0, 3, ps_y)
                emit_scan_sd(1)
                emit_cw(2)
                emit_cw(3)
                emit_scan_front(1, 0)
                emit_scan_front(1, 1)
                staged[(5, "b")] = emit_bc_stage(5, "b")
                staged[(5, "c")] = emit_bc_stage(5, "c")
                emit_bc_proj(4)
                emit_scan_front(1, 2)
                emit_scan_front(1, 3)
                emit_scan_y(1, 0, ps_y)
                emit_scan_y(1, 1, ps_y)
                staged[(6, "b")] = emit_bc_stage(6, "b")
                staged[(6, "c")] = emit_bc_stage(6, "c")
                emit_bc_proj(5)
                emit_scan_y(1, 2, ps_y)
                emit_scan_y(1, 3, ps_y)
                emit_scan_sd(2)
                emit_cw(4)
                emit_cw(5)
                emit_scan_front(2, 0)
                emit_scan_front(2, 1)
                staged[(7, "b")] = emit_bc_stage(7, "b")
                staged[(7, "c")] = emit_bc_stage(7, "c")
                emit_bc_proj(6)
                emit_scan_front(2, 2)
                emit_scan_front(2, 3)
                emit_scan_y(2, 0, ps_y)
                emit_scan_y(2, 1, ps_y)
                emit_cw(6)
                emit_bc_proj(7)
                emit_scan_y(2, 2, ps_y)
                emit_scan_y(2, 3, ps_y)
                emit_scan_sd(3)
                emit_cw(7)
                emit_scan_front(3, 0)
                emit_scan_front(3, 1)
                emit_scan_front(3, 2)
                emit_scan_front(3, 3)
                emit_z_alloc()
                emit_z(0)
                emit_z(1)
                emit_z(2)
                emit_z(3)
                emit_scan_y(3, 0, ps_y)
                emit_z(4)
                emit_z(5)
                emit_scan_y(3, 1, ps_y)
                emit_z(6)
                emit_z(7)
                emit_scan_y(3, 2, ps_y)
                emit_z(8)
                emit_z(9)
                emit_z(10)
                emit_z(11)
                emit_scan_y(3, 3, ps_y)
                emit_z(12)
                emit_z(13)
                emit_z(14)
                emit_z(15)

                # ---------------- tail: zT + logits ---------------------
                zT = cst.tile([P, 2 * T], dt.bfloat16, tag="zT")  # (u, t)
                for ut in range(2):
                    if with_b_yo:
                        nc.scalar.activation(
                            zT[:, ut * T : (ut + 1) * T], zps[ut][:], AF.Relu,
                            bias=b_yo[:, ut : ut + 1],
                        )
                    else:
                        nc.scalar.activation(
                            zT[:, ut * T : (ut + 1) * T], zps[ut][:], AF.Relu
                        )

                logit = cst.tile([P, NCH * A], dt.float32, tag="logit")
                for tt in range(NCH):
                    ps = ps_y.tile([P, A], dt.float32, tag="y", name=f"lg{tt}")
                    nmm = 3 if with_b_head else 2
                    for ut in range(2):
                        nc.tensor.matmul(
                            ps[:],
                            zT[:, ut * T + tt * P : ut * T + (tt + 1) * P],
                            w_hd[:, ut * A : (ut + 1) * A],
                            start=(ut == 0),
                            stop=(ut == nmm - 1),
                        )
                    if with_b_head:
                        nc.tensor.matmul(
                            ps[:],
                            ones_bf[0:1, tt * P : (tt + 1) * P],
                            b_hd[:],
                            start=False,
                            stop=True,
                        )
                    nc.scalar.activation(logit[:, tt * A : (tt + 1) * A], ps[:], AF.Copy)
                    nc.sync.dma_start(
                        out=out_e[tt * P : (tt + 1) * P, :],
                        in_=logit[:, tt * A : (tt + 1) * A],
                    )

    _split_multi_waits(nc)
    return nc


def kernel(obs, W_in, b_in, A_log, dt_bias, W_dt, W_B, W_C, W_yo, b_yo, W_head, b_head):
    _inject_axon_hooks()
    _patch_tile()
    from concourse.bass_utils import run_bass_kernel_spmd

    obs = np.asarray(obs, dtype=np.float32)
    flags = (
        bool(np.any(np.asarray(b_in) != 0)),
        bool(np.any(np.asarray(b_yo) != 0)),
        bool(np.any(np.asarray(b_head) != 0)),
    )
    # First call: build once (the verified path). Repeat calls in one
    # process rebuild a fresh graph — re-executing a previously-run nc with
    # new inputs has crashed the exec unit (NRT status 101) in testing.
    if flags not in _CACHE:
        _CACHE[flags] = _build(*flags)
    elif _EXECUTED.get(flags):
        _CACHE[flags] = _build(*flags)
    nc = _CACHE[flags]
    _EXECUTED[flags] = True

    obsT = obs.reshape(T, BSZ, OBSD).transpose(1, 2, 0)  # (B, 256, T)

    def colblocks(w, blk):
        # (D, M) -> (P, M//blk * KT * blk): per-partition contiguous blocks
        m = w.shape[1]
        return np.ascontiguousarray(
            w.reshape(KT, P, m // blk, blk).transpose(1, 2, 0, 3).reshape(P, -1)
        )

    base = {
        "w_in": np.ascontiguousarray(W_in).astype(BF16),
        "w_dt": colblocks(np.asarray(W_dt), H).astype(BF16),
        "w_b": colblocks(np.asarray(W_B), P).astype(BF16),
        "w_c": colblocks(np.asarray(W_C), P).astype(BF16),
        "w_yo": colblocks(np.asarray(W_yo), U).astype(BF16),
        "w_hd": np.ascontiguousarray(
            np.asarray(W_head).reshape(2, P, A).transpose(1, 0, 2).reshape(P, 2 * A)
        ).astype(BF16),
        "neg_a": (-np.exp(np.asarray(A_log, np.float64)))
        .astype(np.float32)
        .reshape(H, 1),
        "dtb": np.asarray(dt_bias, np.float32).reshape(H, 1),
    }
    if flags[0]:
        base["b_in"] = np.ascontiguousarray(
            np.asarray(b_in, np.float32).reshape(KT, P).T
        )
    if flags[1]:
        base["b_yo"] = np.ascontiguousarray(
            np.asarray(b_yo, np.float32).reshape(2, P).T
        )
    if flags[2]:
        base["b_hd"] = np.asarray(b_head).astype(BF16).reshape(1, A)
    in_maps = [
        dict(base, obsT=np.ascontiguousarray(obsT[c]).astype(BF16)) for c in range(BSZ)
    ]
    global _last_in_maps
    _last_in_maps = in_maps
    res = run_bass_kernel_spmd(nc, in_maps, core_ids=list(range(BSZ)))
    out = np.stack([res.results[c]["out"] for c in range(BSZ)], axis=1)
    return out.astype(np.float32)
